# revision 1
# baseline (speedup 1.0000x reference)
"""Trainium2 Bass kernel for nn_BackupBarrierCBF.

Reference semantics (B=1024, A=64, T=50 unicycle rollout + rect-vs-disc
distance + min-over-horizon + saturation). Crucial subtleties:
  - braking controller: u = (-9*tanh(2*v), 0) => theta is CONSTANT, so
    positions are x0 + cos(theta)*dt*cumsum(v).
  - veh_veh_distance receives traj[..., 0:3] = (x, y, v): the body-frame
    rotation angle is the (time-varying) VELOCITY, not theta.
  - traj slot k holds the state AFTER k+1 steps: position cumsum uses
    v_0..v_k while the stored rotation angle is v_{k+1}.

Per-core structure (batch rows on the 128 partitions):
  - 50-step serial v-recurrence (ACT Tanh + DVE scalar_tensor_tensor)
    writing straight into a t-major trajectory (all chain ops contiguous);
    the col-major cumsum ST is built by per-step adds and the angle range
    reduction runs in the rollout's DVE slack. Constants precede the
    rollout so their ACT Sins/Sqrts don't thrash the Tanh table.
  - sin/cos of v(t) on ACT with col-major STRIDED writes (2.2x ACT penalty,
    but ACT has slack and every later DVE op stays unit-stride). Range
    reduction only for the first k_red slots (|v| provably <= pi afterward:
    while |v|>2.2 each step shrinks |v| by >= 0.8997 and the map keeps
    |v| <= pi once below). cos x = sin(pi/2 - |x|).
  - distance phase: ~28 big [128, 64, 50] DVE ops, a-major, unit inner
    stride, per-agent constants broadcast with 0-step APs; SINV-products
    ordered first (COSV finishes later on ACT); abs on ACT, fine-grained.
  - NO gpsimd tensor work: gpsimd shares the DVE SBUF port (measured 2.5x
    DVE slowdown when overlapped - net zero).

Sharding: pure data parallel over batch B across 8 cores (128 rows/core).
"""
import numpy as np
import concourse.bass as bass
import concourse.bacc as bacc
import concourse.tile as tile
from concourse import mybir
from concourse.bass_utils import run_bass_kernel_spmd

F32 = mybir.dt.float32
I32 = mybir.dt.int32
OP = mybir.AluOpType
ACT = mybir.ActivationFunctionType

B, A, F = 1024, 64, 15
N_CORES = 8
PB = B // N_CORES          # 128 batch rows per core (partition dim)
T = 50
NC2 = 2 * A                # 128 columns: [ego agents | other agents]
NT = T * A                 # 3200
TWO_PI = float(2.0 * np.pi)

_cache: dict = {}


def _ap(t: bass.AP, extra_offset: int, free_dims: list) -> bass.AP:
    """View into tile t: keep partition dim, replace free dims."""
    return bass.AP(tensor=t.tensor, offset=t.offset + extra_offset,
                   ap=[list(t.ap[0])] + [list(d) for d in free_dims])


def _build(dt_uniform, k_red):
    nc = bacc.Bacc("TRN2", target_bir_lowering=False)
    data = nc.dram_tensor("data", [PB, A * F], F32, kind="ExternalInput")
    out = nc.dram_tensor("out", [PB, A], F32, kind="ExternalOutput")

    with tile.TileContext(nc) as tc:
        with tc.tile_pool(name="pool", bufs=1) as pool:
            # ---------------- load ----------------
            D = pool.tile([PB, A * F], F32)
            nc.sync.dma_start(out=D[:], in_=data[:])

            def fld(k):  # [128, 64] strided view of per-agent field k
                return _ap(D, k, [[F, A]])

            halfpi = pool.tile([PB, 1], F32)
            nc.vector.memset(halfpi[:], float(np.pi / 2))

            cons = pool.tile([PB, 12, A], F32)

            def c(i):
                return _ap(cons, i * A, [[1, A]])

            def cb(i):  # broadcast over inner t: [128, 64, T]
                return _ap(cons, i * A, [[1, A], [0, T]])

            C_P0X, C_P0Y = 0, 1
            C_D1, C_D2, C_D3, C_K2Y = 2, 3, 4, 5
            C_CEDT, C_SEDT, C_CADT, C_SADT = 6, 7, 8, 9
            C_RE, C_RA = 10, 11

            scr = pool.tile([PB, 10, A], F32)

            def s(i):
                return _ap(scr, i * A, [[1, A]])

            ki = pool.tile([PB, 4, A], I32)

            def kis(i):
                return _ap(ki, i * A, [[1, A]])

            # ---------------- per-agent constants (front) ------------
            # Their ACT Sins/Sqrts run before any Tanh so the ACT table is
            # loaded once per function; four separate scratches keep the
            # sincos pipelines independent.
            def sincos(theta_ap, out_sin, out_cos, base):
                for idx, (want_cos, dst) in enumerate(((False, out_sin),
                                                       (True, out_cos))):
                    sc = s(base + idx)
                    shift = 0.25 if want_cos else 0.0
                    nc.vector.tensor_scalar(out=sc, in0=theta_ap,
                                            scalar1=1.0 / TWO_PI, scalar2=shift,
                                            op0=OP.mult, op1=OP.add)
                    nc.vector.tensor_copy(out=kis(base + idx), in_=sc)
                    nc.vector.tensor_copy(out=sc, in_=kis(base + idx))
                    nc.vector.scalar_tensor_tensor(
                        out=sc, in0=sc, scalar=-TWO_PI, in1=theta_ap,
                        op0=OP.mult, op1=OP.add)
                    nc.scalar.activation(
                        out=dst, in_=sc, func=ACT.Sin,
                        bias=halfpi[:] if want_cos else 0.0, scale=1.0)

            sincos(fld(7), c(C_SADT), c(C_CADT), 0)
            sincos(fld(3), c(C_SEDT), c(C_CEDT), 2)
            for i in (C_CADT, C_SADT, C_CEDT, C_SEDT):
                nc.vector.tensor_mul(out=c(i), in0=c(i), in1=fld(14))

            nc.vector.tensor_mul(out=s(4), in0=fld(8), in1=fld(8))
            nc.vector.tensor_mul(out=s(5), in0=fld(9), in1=fld(9))
            nc.vector.tensor_add(out=s(4), in0=s(4), in1=s(5))
            nc.scalar.activation(out=c(C_RE), in_=s(4), func=ACT.Sqrt,
                                 scale=0.25)
            nc.vector.tensor_mul(out=s(6), in0=fld(11), in1=fld(11))
            nc.vector.tensor_mul(out=s(7), in0=fld(12), in1=fld(12))
            nc.vector.tensor_add(out=s(6), in0=s(6), in1=s(7))
            nc.scalar.activation(out=c(C_RA), in_=s(6), func=ACT.Sqrt,
                                 scale=0.25)
            # d1 = 0.5*(We-Le); d2 = 0.5*(Wa-La); k1y = 0.5*We+ra;
            # k2y = 0.5*Wa+re; d3 = k2y-k1y
            nc.vector.tensor_sub(out=s(8), in0=fld(9), in1=fld(8))
            nc.vector.tensor_scalar_mul(out=c(C_D1), in0=s(8), scalar1=0.5)
            nc.vector.tensor_sub(out=s(9), in0=fld(12), in1=fld(11))
            nc.vector.tensor_scalar_mul(out=c(C_D2), in0=s(9), scalar1=0.5)
            nc.vector.scalar_tensor_tensor(
                out=s(8), in0=fld(9), scalar=0.5, in1=c(C_RA),
                op0=OP.mult, op1=OP.add)          # k1y
            nc.vector.scalar_tensor_tensor(
                out=c(C_K2Y), in0=fld(12), scalar=0.5, in1=c(C_RE),
                op0=OP.mult, op1=OP.add)          # k2y
            nc.vector.tensor_sub(out=c(C_D3), in0=c(C_K2Y), in1=s(8))
            nc.vector.tensor_sub(out=c(C_P0X), in0=fld(4), in1=fld(0))
            nc.vector.tensor_sub(out=c(C_P0Y), in0=fld(5), in1=fld(1))

            # ---------------- rollout ----------------
            # Serial chain writes straight into t-major VT (slot j at
            # j*NC2); col-major ST built by per-step adds; the angle
            # range-reduce fills the rollout's DVE slack.
            VT = pool.tile([PB, (T + 1) * NC2], F32, tag="tVT")
            ST = pool.tile([PB, NC2 * T], F32, tag="tST")

            def vslot(j):  # j=0: strided input view; j in 1..50: contiguous
                if j == 0:
                    return _ap(D, 2, [[4, 2], [F, A]])
                return _ap(VT, j * NC2, [[1, NC2]])

            def stslot(k):  # k in 0..49, col-major strided
                return _ap(ST, k, [[T, NC2]])

            G = pool.tile([PB, NC2], F32)
            nc.vector.tensor_copy(out=stslot(0), in_=vslot(0))

            if dt_uniform is None:
                NDT2 = pool.tile([PB, NC2], F32)
                nc.vector.tensor_scalar_mul(
                    out=NDT2[:], in0=_ap(D, 14, [[0, 2], [F, A]]), scalar1=-9.0)

            MS = KI2 = None
            if k_red > 0:
                MS = pool.tile([PB, NC2 * k_red], F32, tag="tPXY")
                KI2 = pool.tile([PB, NC2 * k_red], I32, tag="tSCR")

            SEv = _ap(ST, 0, [[T, A], [1, T]])
            SAv = _ap(ST, A * T, [[T, A], [1, T]])
            PXY = pool.tile([PB, 2 * NT], F32, tag="tPXY")
            SCR = pool.tile([PB, 2 * NT], F32, tag="tSCR")

            for j in range(1, T + 1):
                nc.scalar.activation(out=G[:], in_=vslot(j - 1),
                                     func=ACT.Tanh, scale=2.0)
                if dt_uniform is None:
                    nc.vector.tensor_mul(out=G[:], in0=G[:], in1=NDT2[:])
                    nc.vector.tensor_add(out=vslot(j), in0=vslot(j - 1),
                                         in1=G[:])
                else:
                    nc.vector.scalar_tensor_tensor(
                        out=vslot(j), in0=G[:], scalar=-9.0 * float(dt_uniform),
                        in1=vslot(j - 1), op0=OP.mult, op1=OP.add)
                if j < T:
                    nc.vector.tensor_add(out=stslot(j), in0=stslot(j - 1),
                                         in1=vslot(j))
                if j == k_red and k_red > 0:
                    # range-reduce angle slots 1..k_red in place (all
                    # ST-adds reading the raw values already emitted)
                    red_view = _ap(VT, NC2, [[1, NC2 * k_red]])
                    nc.vector.tensor_scalar_mul(out=MS[:], in0=red_view,
                                                scalar1=1.0 / TWO_PI)
                    nc.vector.tensor_copy(out=KI2[:], in_=MS[:])
                    nc.vector.tensor_copy(out=MS[:], in_=KI2[:])
                    nc.vector.scalar_tensor_tensor(
                        out=red_view, in0=MS[:], scalar=-TWO_PI, in1=red_view,
                        op0=OP.mult, op1=OP.add)

            # ---------------- trig of v (angles are v_{k+1}) ----------
            # t-major contiguous reads, col-major strided writes (ACT has
            # slack; DVE consumers stay unit-stride).  SINV first so the
            # rel phase's SINV-products can start earliest.
            ang = _ap(VT, NC2, [[1, T * NC2]])
            SINV = pool.tile([PB, NC2 * T], F32)
            COSV = pool.tile([PB, NC2 * T], F32)
            cm_out_sin = _ap(SINV, 0, [[1, T], [T, NC2]])
            cm_out_cos = _ap(COSV, 0, [[1, T], [T, NC2]])
            nc.scalar.activation(out=cm_out_sin, in_=ang, func=ACT.Sin)
            nc.scalar.activation(out=cm_out_cos, in_=ang, func=ACT.Abs)
            nc.scalar.activation(out=COSV[:], in_=COSV[:], func=ACT.Sin,
                                 bias=halfpi[:], scale=-1.0)

            S1 = _ap(SCR, 0, [[1, NT]])
            S2 = _ap(SCR, NT, [[1, NT]])
            PX = _ap(PXY, 0, [[1, NT]])
            PY = _ap(PXY, NT, [[1, NT]])

            nc.vector.tensor_mul(out=S1, in0=SAv, in1=cb(C_CADT))
            nc.vector.tensor_add(out=S1, in0=S1, in1=cb(C_P0X))
            nc.vector.tensor_mul(out=S2, in0=SEv, in1=cb(C_CEDT))
            nc.vector.tensor_sub(out=PX, in0=S1, in1=S2)
            nc.vector.tensor_mul(out=S1, in0=SAv, in1=cb(C_SADT))
            nc.vector.tensor_add(out=S1, in0=S1, in1=cb(C_P0Y))
            nc.vector.tensor_mul(out=S2, in0=SEv, in1=cb(C_SEDT))
            nc.vector.tensor_sub(out=PY, in0=S1, in1=S2)

            # ---------------- body-frame components ----------------
            # SINV-products first (COSV lands later on ACT).
            CE = _ap(COSV, 0, [[1, NT]])
            CA = _ap(COSV, NT, [[1, NT]])
            SE_ = _ap(SINV, 0, [[1, NT]])
            SA_ = _ap(SINV, NT, [[1, NT]])
            R12 = pool.tile([PB, 2 * NT], F32, tag="tST")
            R1X = _ap(R12, 0, [[1, NT]])
            R1Y = _ap(R12, NT, [[1, NT]])
            R34 = pool.tile([PB, 2 * NT], F32)
            R2X = _ap(R34, 0, [[1, NT]])
            R2Y = _ap(R34, NT, [[1, NT]])

            nc.vector.tensor_mul(out=R1X, in0=SE_, in1=PY)
            nc.vector.tensor_mul(out=R1Y, in0=SE_, in1=PX)
            nc.vector.tensor_mul(out=R2X, in0=SA_, in1=PY)
            nc.vector.tensor_mul(out=R2Y, in0=SA_, in1=PX)
            nc.vector.tensor_mul(out=S1, in0=CE, in1=PX)
            nc.vector.tensor_add(out=R1X, in0=R1X, in1=S1)   # rel1x
            nc.vector.tensor_mul(out=S2, in0=CE, in1=PY)
            nc.vector.tensor_sub(out=R1Y, in0=S2, in1=R1Y)   # rel1y
            nc.vector.tensor_mul(out=S1, in0=CA, in1=PX)
            nc.vector.tensor_add(out=R2X, in0=R2X, in1=S1)   # -rel2x; |.| ok
            nc.vector.tensor_mul(out=S2, in0=CA, in1=PY)
            nc.vector.tensor_sub(out=R2Y, in0=R2Y, in1=S2)   # rel2y

            # |rel| on ACT, then the shifted max-tree:
            # dist = max(max(|r1x|+d1, |r1y|) + d3, max(|r2x|+d2, |r2y|)) - k2y
            # with d1=k1y-k1x, d2=k2y-k2x, d3=k2y-k1y; -k2y lands after the
            # min-reduce as a [128,64] op (k's are constant over t).
            for R in (R1X, R1Y, R2X, R2Y):
                nc.scalar.activation(out=R, in_=R, func=ACT.Abs)
            nc.vector.tensor_add(out=R1X, in0=R1X, in1=cb(C_D1))
            nc.vector.tensor_tensor(out=R1X, in0=R1X, in1=R1Y, op=OP.max)
            nc.vector.tensor_add(out=R2X, in0=R2X, in1=cb(C_D2))
            nc.vector.tensor_tensor(out=R2X, in0=R2X, in1=R2Y, op=OP.max)
            nc.vector.tensor_add(out=R1X, in0=R1X, in1=cb(C_D3))
            nc.vector.tensor_tensor(out=R1X, in0=R1X, in1=R2X, op=OP.max)

            H = pool.tile([PB, A], F32)
            nc.vector.tensor_reduce(out=H[:],
                                    in_=_ap(R12, 0, [[T, A], [1, T]]),
                                    axis=mybir.AxisListType.X, op=OP.min)
            nc.vector.tensor_sub(out=H[:], in0=H[:], in1=c(C_K2Y))
            OUTT = pool.tile([PB, A], F32)
            nc.scalar.activation(out=H[:], in_=H[:], func=ACT.Tanh, scale=0.1)
            nc.vector.tensor_scalar_mul(out=OUTT[:], in0=H[:], scalar1=5.0)
            nc.sync.dma_start(out=out[:], in_=OUTT[:])

    nc.compile()
    return nc


def _get_nc(dt_uniform, k_red):
    key = ("nc", dt_uniform, k_red)
    if key not in _cache:
        _cache[key] = _build(dt_uniform, k_red)
    return _cache[key]


def _make_runner(nc):
    """One-time build of a cached jitted SPMD executable for nc (the
    equivalent of bass2jax.run_bass_via_pjrt, but reusable across calls so
    repeated kernel() invocations skip the jax retrace)."""
    import jax
    from jax.sharding import Mesh, PartitionSpec
    from jax.experimental.shard_map import shard_map
    from concourse import bass2jax, mybir as _mybir

    bass2jax.install_neuronx_cc_hook()
    partition_name = (nc.partition_id_tensor.name
                      if nc.partition_id_tensor else None)
    in_names, out_names, out_avals, zero_outs = [], [], [], []
    for alloc in nc.m.functions[0].allocations:
        if not isinstance(alloc, _mybir.MemoryLocationSet):
            continue
        name = alloc.memorylocations[0].name
        if alloc.kind == "ExternalInput":
            if name != partition_name:
                in_names.append(name)
        elif alloc.kind == "ExternalOutput":
            shape = tuple(alloc.tensor_shape)
            dtype = _mybir.dt.np(alloc.dtype)
            out_names.append(name)
            out_avals.append(jax.core.ShapedArray(shape, dtype))
            zero_outs.append(np.zeros(shape, dtype))
    n_params = len(in_names)
    all_names = in_names + out_names
    if partition_name is not None:
        all_names = all_names + [partition_name]
    donate = tuple(range(n_params, n_params + len(out_names)))

    def _body(*args):
        operands = list(args)
        if partition_name is not None:
            operands.append(bass2jax.partition_id_tensor())
        outs = bass2jax._bass_exec_p.bind(
            *operands, out_avals=tuple(out_avals), in_names=tuple(all_names),
            out_names=tuple(out_names), lowering_input_output_aliases=(),
            sim_require_finite=True, sim_require_nnan=True, nc=nc)
        return tuple(outs)

    mesh = Mesh(np.asarray(jax.devices()[:N_CORES]), ("core",))
    in_specs = (PartitionSpec("core"),) * (n_params + len(out_names))
    out_specs = (PartitionSpec("core"),) * len(out_names)
    sharded = jax.jit(
        shard_map(_body, mesh=mesh, in_specs=in_specs, out_specs=out_specs,
                  check_rep=False),
        donate_argnums=donate, keep_unused=True)
    concat_zeros = [np.zeros((N_CORES * z.shape[0], *z.shape[1:]), z.dtype)
                    for z in zero_outs]

    def run(full_data_2d):  # [B, A*F] -> [B, A]
        outs = sharded(full_data_2d, *[z.copy() for z in concat_zeros])
        return np.asarray(outs[out_names.index("out")])

    return run


def _params_for(data: np.ndarray):
    dt = data[..., 14]
    dt0 = float(dt.flat[0])
    dt_uniform = dt0 if bool(np.all(dt == dt0)) else None
    vmax = float(np.abs(data[..., [2, 6]]).max())
    # slots j >= k_red have |v_j| <= pi: while |v| > 2.2 each step shrinks
    # |v| by >= 9*dt_min*tanh(4.4), and the map keeps |v| <= pi once below
    # (valid when the max step 9*dt_max <= pi; otherwise reduce every slot).
    dt_min = float(dt.min())
    dt_max = float(dt.max())
    shrink = 9.0 * dt_min * 0.9997
    if 9.0 * dt_max > np.pi or shrink <= 1e-6:
        k_red = T
    else:
        k_red = int(min(T, max(0, np.ceil((vmax - np.pi) / shrink) + 1)))
    return dt_uniform, k_red


def _run(data: np.ndarray, trace: bool = False):
    data = np.ascontiguousarray(data, dtype=np.float32)
    assert data.shape == (B, A, F), data.shape
    dt_uniform, k_red = _params_for(data)
    nc = _get_nc(dt_uniform, k_red)
    in_maps = [{"data": data[c * PB:(c + 1) * PB].reshape(PB, A * F)}
               for c in range(N_CORES)]
    res = run_bass_kernel_spmd(nc, in_maps, core_ids=list(range(N_CORES)),
                               trace=trace)
    full = np.concatenate([res.results[c]["out"] for c in range(N_CORES)],
                          axis=0)
    return full, res


def kernel(data: np.ndarray) -> np.ndarray:
    data = np.ascontiguousarray(data, dtype=np.float32)
    assert data.shape == (B, A, F), data.shape
    dt_uniform, k_red = _params_for(data)
    key = ("runner", dt_uniform, k_red)
    if key not in _cache:
        _cache[key] = _make_runner(_get_nc(dt_uniform, k_red))
    return _cache[key](data.reshape(B, A * F)).astype(np.float32)



# revision 2
# speedup vs baseline: 1.0697x; 1.0697x over previous
"""Trainium2 Bass kernel for nn_BackupBarrierCBF — v3.

v2 (121.8us) + rollout restructure:
  - Serial ACT-Tanh chain only until |v| provably <= 0.2 (k_exact ~ 18
    steps from vmax~15; host-verified on a dense grid). The remaining
    TAIL = 50-k_exact slots are VECTORIZED: v_{k+m} = w*FV_m(w^2) and
    S_{k+m} = S_k + w*FS_m(w^2) with per-slot deg-1 polynomial fits in
    u = w^2 computed on host (max fit err ~1.3e-3 on v, 7e-4 on S —
    fp16-noise level). Coefficients ship pre-broadcast as an extra
    fp16 DMA input (hidden behind the serial phase).
  - Tail angles |v|<=0.2: no range reduction, no abs pass for cos.
  - Constants phase: single batched range-reduce + one Sin over [4A]
    (shift-fold trick for the cos halves), one Sqrt over [2A]; all DVE
    prep interleaved into the chain's DVE gaps; ACT ops after the chain
    (no Sin/Tanh table thrash).
  - STH convert on DVE post-chain (off the ACT trig path).
Distance phase identical to v2 (fp16 t-major, 2x DVE).
"""
import numpy as np
import concourse.bass as bass
import concourse.bacc as bacc
import concourse.tile as tile
from concourse import mybir
from concourse.bass_utils import run_bass_kernel_spmd

F32 = mybir.dt.float32
F16 = mybir.dt.float16
I32 = mybir.dt.int32
OP = mybir.AluOpType
ACT = mybir.ActivationFunctionType

B, A, F = 1024, 64, 15
N_CORES = 8
PB = B // N_CORES
T = 50
NC2 = 2 * A
NT = T * A
TWO_PI = float(2.0 * np.pi)
MAGIC = float(1.5 * 2 ** 23)   # fp32 round-to-nearest-even bias

_cache: dict = {}


def _ap(t: bass.AP, extra_offset: int, free_dims: list) -> bass.AP:
    return bass.AP(tensor=t.tensor, offset=t.offset + extra_offset,
                   ap=[list(t.ap[0])] + [list(d) for d in free_dims])


def _build(dt_uniform, k_red, k_exact):
    tail = T - k_exact
    nc = bacc.Bacc("TRN2", target_bir_lowering=False)
    # field-major input [PB, F*A]: field k occupies cols [k*A, (k+1)*A)
    data = nc.dram_tensor("data2", [PB, F * A], F32, kind="ExternalInput")
    # duplicate of the two v columns; tiny DMA so the chain starts early
    dvin = nc.dram_tensor("dv", [PB, 2 * A], F32, kind="ExternalInput")
    coef = (nc.dram_tensor("coef", [PB, 4 * tail * NC2], F16,
                           kind="ExternalInput") if tail > 0 else None)
    out = nc.dram_tensor("out", [PB, A], F32, kind="ExternalOutput")

    with tile.TileContext(nc) as tc:
        with tc.tile_pool(name="pool", bufs=1) as pool:
            Dv = pool.tile([PB, 2 * A], F32)
            nc.sync.dma_start(out=Dv[:], in_=dvin[:])
            D = pool.tile([PB, F * A], F32)
            nc.sync.dma_start(out=D[:], in_=data[:])
            CO = None
            if tail > 0:
                CO = pool.tile([PB, 4 * tail * NC2], F16)
                nc.sync.dma_start(out=CO[:], in_=coef[:])

            def fld(k):
                return _ap(D, k * A, [[1, A]])

            halfpi = pool.tile([PB, 1], F32)
            cons = pool.tile([PB, 12, A], F32)
            consh = pool.tile([PB, 12, A], F16)
            scr = pool.tile([PB, 10, A], F32)
            ki = pool.tile([PB, 4, A], I32)

            def c(i):
                return _ap(cons, i * A, [[1, A]])

            def cbh(i):
                return _ap(consh, i * A, [[0, T], [1, A]])

            def s(i):
                return _ap(scr, i * A, [[1, A]])

            C_P0X, C_P0Y = 0, 1
            C_D1, C_D2, C_D3, C_K2Y = 2, 3, 4, 5
            C_CEDT, C_SEDT, C_CADT, C_SADT = 6, 7, 8, 9
            C_RE, C_RA = 10, 11

            # ---- constants prep (pure DVE), interleaved into the chain ----
            # Batched sincos args: scr rows 0..3 hold args for
            # [cos(th_e), sin(th_e), cos(th_a), sin(th_a)] -> cons 6..9.
            prep = []
            S4 = _ap(scr, 0, [[1, 4 * A]])
            KI4 = _ap(ki, 0, [[1, 4 * A]])
            prep.append(lambda: nc.vector.memset(halfpi[:], float(np.pi / 2)))
            prep.append(lambda: nc.vector.tensor_copy(
                out=_ap(scr, 0, [[A, 2], [1, A]]),
                in_=_ap(D, 3 * A, [[0, 2], [1, A]])))
            prep.append(lambda: nc.vector.tensor_copy(
                out=_ap(scr, 2 * A, [[A, 2], [1, A]]),
                in_=_ap(D, 7 * A, [[0, 2], [1, A]])))
            # sc = th/2pi (+0.25 on the cos rows 0 and 2), then magic-number
            # round-to-nearest (adding 1.5*2^23 snaps the mantissa to ulp=1)
            prep.append(lambda: nc.vector.tensor_scalar(
                out=S4, in0=S4, scalar1=1.0 / TWO_PI, scalar2=0.0,
                op0=OP.mult, op1=OP.add))
            prep.append(lambda: nc.vector.tensor_scalar(
                out=_ap(scr, 0, [[2 * A, 2], [1, A]]),
                in0=_ap(scr, 0, [[2 * A, 2], [1, A]]),
                scalar1=1.0, scalar2=0.25, op0=OP.mult, op1=OP.add))
            prep.append(lambda: nc.vector.tensor_scalar(
                out=S4, in0=S4, scalar1=MAGIC, scalar2=MAGIC,
                op0=OP.add, op1=OP.subtract))
            # subtract the shift back on cos rows: k - 0.25
            prep.append(lambda: nc.vector.tensor_scalar(
                out=_ap(scr, 0, [[2 * A, 2], [1, A]]),
                in0=_ap(scr, 0, [[2 * A, 2], [1, A]]),
                scalar1=1.0, scalar2=-0.25, op0=OP.mult, op1=OP.add))
            # arg = th - 2pi*k  (th re-read strided)
            prep.append(lambda: nc.vector.scalar_tensor_tensor(
                out=_ap(scr, 0, [[1, A]]), in0=_ap(scr, 0, [[1, A]]),
                scalar=-TWO_PI, in1=fld(3), op0=OP.mult, op1=OP.add))
            prep.append(lambda: nc.vector.scalar_tensor_tensor(
                out=_ap(scr, A, [[1, A]]), in0=_ap(scr, A, [[1, A]]),
                scalar=-TWO_PI, in1=fld(3), op0=OP.mult, op1=OP.add))
            prep.append(lambda: nc.vector.scalar_tensor_tensor(
                out=_ap(scr, 2 * A, [[1, A]]), in0=_ap(scr, 2 * A, [[1, A]]),
                scalar=-TWO_PI, in1=fld(7), op0=OP.mult, op1=OP.add))
            prep.append(lambda: nc.vector.scalar_tensor_tensor(
                out=_ap(scr, 3 * A, [[1, A]]), in0=_ap(scr, 3 * A, [[1, A]]),
                scalar=-TWO_PI, in1=fld(7), op0=OP.mult, op1=OP.add))
            # sumsq for r_e, r_a into scr rows 4 (ego) and 5 (agent)
            prep.append(lambda: nc.vector.tensor_mul(
                out=s(4), in0=fld(8), in1=fld(8)))
            prep.append(lambda: nc.vector.tensor_mul(
                out=s(6), in0=fld(9), in1=fld(9)))
            prep.append(lambda: nc.vector.tensor_add(
                out=s(4), in0=s(4), in1=s(6)))
            prep.append(lambda: nc.vector.tensor_mul(
                out=s(5), in0=fld(11), in1=fld(11)))
            prep.append(lambda: nc.vector.tensor_mul(
                out=s(6), in0=fld(12), in1=fld(12)))
            prep.append(lambda: nc.vector.tensor_add(
                out=s(5), in0=s(5), in1=s(6)))
            # d1/d2/p0
            prep.append(lambda: nc.vector.tensor_sub(
                out=s(6), in0=fld(9), in1=fld(8)))
            prep.append(lambda: nc.vector.tensor_scalar_mul(
                out=c(C_D1), in0=s(6), scalar1=0.5))
            prep.append(lambda: nc.vector.tensor_sub(
                out=s(7), in0=fld(12), in1=fld(11)))
            prep.append(lambda: nc.vector.tensor_scalar_mul(
                out=c(C_D2), in0=s(7), scalar1=0.5))
            prep.append(lambda: nc.vector.tensor_sub(
                out=c(C_P0X), in0=fld(4), in1=fld(0)))
            prep.append(lambda: nc.vector.tensor_sub(
                out=c(C_P0Y), in0=fld(5), in1=fld(1)))

            # ---------------- rollout: serial head ----------------
            VT = pool.tile([PB, (k_exact + 1) * NC2], F32)
            ST = pool.tile([PB, (k_exact + 1) * NC2], F32)
            STH = pool.tile([PB, T * NC2], F16)
            G = pool.tile([PB, NC2], F32)

            def vslot(j):
                if j == 0:
                    return _ap(Dv, 0, [[A, 2], [1, A]])
                return _ap(VT, j * NC2, [[1, NC2]])

            def stslot(k):
                return _ap(ST, k * NC2, [[1, NC2]])

            nc.vector.tensor_copy(out=stslot(0), in_=vslot(0))

            NDT2 = None
            if dt_uniform is None:
                NDT2 = pool.tile([PB, NC2], F32)
                nc.vector.tensor_scalar_mul(
                    out=NDT2[:], in0=_ap(D, 14 * A, [[0, 2], [1, A]]),
                    scalar1=-9.0)

            # Interleave preps into the chain from step 4 on (2 per step):
            # by then the big D DMA has landed, so a prep never stalls the
            # chain's STTs (emitting preps after the loop would let the
            # scheduler place all of them — waiting on D — ahead of STT1).
            pi = 0
            for j in range(1, k_exact + 1):
                nc.scalar.activation(out=G[:], in_=vslot(j - 1),
                                     func=ACT.Tanh, scale=2.0)
                if dt_uniform is None:
                    nc.vector.tensor_mul(out=G[:], in0=G[:], in1=NDT2[:])
                    nc.vector.tensor_add(out=vslot(j), in0=vslot(j - 1),
                                         in1=G[:])
                else:
                    nc.vector.scalar_tensor_tensor(
                        out=vslot(j), in0=G[:], scalar=-9.0 * float(dt_uniform),
                        in1=vslot(j - 1), op0=OP.mult, op1=OP.add)
                nc.vector.tensor_add(out=stslot(j), in0=stslot(j - 1),
                                     in1=vslot(j))
                if j >= 4:
                    for _ in range(2):
                        if pi < len(prep):
                            prep[pi]()
                            pi += 1
            while pi < len(prep):
                prep[pi]()
                pi += 1
            # chain-end gate: z = 0 * v_k, then += 0 on the ACT-const
            # inputs so Sqrt/Sin (and their table loads) cannot be
            # scheduled into the middle of the Tanh chain.
            zg = pool.tile([PB, 1], F32)
            nc.vector.tensor_scalar_mul(out=zg[:],
                                        in0=_ap(VT, k_exact * NC2, [[1, 1]]),
                                        scalar1=0.0)
            nc.vector.tensor_add(out=_ap(scr, 0, [[NC2, 2], [1, NC2]]),
                                 in0=_ap(scr, 0, [[NC2, 2], [1, NC2]]),
                                 in1=_ap(zg, 0, [[0, 2], [0, NC2]]))
            nc.vector.tensor_add(out=_ap(scr, 4 * A, [[A, 2], [1, A]]),
                                 in0=_ap(scr, 4 * A, [[A, 2], [1, A]]),
                                 in1=_ap(zg, 0, [[0, 2], [0, A]]))

            # ---- range-reduce head angle slots 1..k_red (in place) ----
            # magic-number RNE round: k = (x/2pi + M) - M, both ts ops 2x
            if k_red > 0:
                MS = pool.tile([PB, NC2 * k_red], F32, tag="tPXY")
                red_view = _ap(VT, NC2, [[1, NC2 * k_red]])
                nc.vector.tensor_scalar(out=MS[:], in0=red_view,
                                        scalar1=1.0 / TWO_PI, scalar2=MAGIC,
                                        op0=OP.mult, op1=OP.add)
                nc.vector.tensor_scalar(out=MS[:], in0=MS[:],
                                        scalar1=MAGIC, scalar2=-TWO_PI,
                                        op0=OP.subtract, op1=OP.mult)
                nc.vector.tensor_add(out=red_view, in0=red_view, in1=MS[:])

            # ---- ACT constants: one Sqrt over [2A], one Sin over [4A] ----
            nc.scalar.activation(out=_ap(cons, C_RE * A, [[1, 2 * A]]),
                                 in_=_ap(scr, 4 * A, [[1, 2 * A]]),
                                 func=ACT.Sqrt, scale=0.25)
            # gate the Sin batch on the Sqrt output so the scheduler can't
            # interleave Sqrt between the Sin-table trig passes
            zg2 = pool.tile([PB, 1], F32)
            nc.vector.tensor_scalar_mul(out=zg2[:],
                                        in0=_ap(cons, C_RE * A, [[1, 1]]),
                                        scalar1=0.0)
            nc.vector.tensor_add(out=_ap(scr, 0, [[NC2, 2], [1, NC2]]),
                                 in0=_ap(scr, 0, [[NC2, 2], [1, NC2]]),
                                 in1=_ap(zg2, 0, [[0, 2], [0, NC2]]))
            nc.scalar.activation(out=_ap(cons, C_CEDT * A, [[1, 4 * A]]),
                                 in_=S4, func=ACT.Sin)
            # finish constants on DVE (cheap, off chain)
            for i in (C_CADT, C_SADT, C_CEDT, C_SEDT):
                nc.vector.tensor_mul(out=c(i), in0=c(i), in1=fld(14))
            nc.vector.scalar_tensor_tensor(
                out=s(8), in0=fld(9), scalar=0.5, in1=c(C_RA),
                op0=OP.mult, op1=OP.add)          # k1y
            nc.vector.scalar_tensor_tensor(
                out=c(C_K2Y), in0=fld(12), scalar=0.5, in1=c(C_RE),
                op0=OP.mult, op1=OP.add)          # k2y
            nc.vector.tensor_sub(out=c(C_D3), in0=c(C_K2Y), in1=s(8))
            nc.vector.tensor_copy(out=_ap(consh, 0, [[1, 12 * A]]),
                                  in_=_ap(cons, 0, [[1, 12 * A]]))

            # ---------------- vectorized tail ----------------
            VTT = pool.tile([PB, max(1, tail) * NC2], F16)
            wv = pool.tile([PB, NC2], F16)
            uv = pool.tile([PB, NC2], F16)
            s20h = pool.tile([PB, NC2], F16)
            if tail > 0:
                nc.vector.tensor_copy(out=wv[:], in_=vslot(k_exact))
                nc.vector.tensor_mul(out=uv[:], in0=vslot(k_exact),
                                     in1=vslot(k_exact))
                nc.vector.tensor_copy(out=s20h[:], in_=stslot(k_exact))

                def cof(i):  # coef block i in [AV, BV, AS, BS], [tail, NC2]
                    return _ap(CO, i * tail * NC2, [[1, tail * NC2]])

                def cof2(i):
                    return _ap(CO, i * tail * NC2, [[NC2, tail], [1, NC2]])

                ub = _ap(uv, 0, [[0, tail], [1, NC2]])
                wb = _ap(wv, 0, [[0, tail], [1, NC2]])
                s20b = _ap(s20h, 0, [[0, tail], [1, NC2]])
                VTT1 = _ap(VTT, 0, [[1, tail * NC2]])
                VTT2 = _ap(VTT, 0, [[NC2, tail], [1, NC2]])
                # angles: VTT = (BV*u + AV) * w
                nc.vector.tensor_mul(out=VTT2, in0=cof2(1), in1=ub)
                nc.vector.tensor_add(out=VTT1, in0=VTT1, in1=cof(0))
                nc.vector.tensor_mul(out=VTT2, in0=VTT2, in1=wb)
                # cumsum: STH[k_exact..] = (BS*u + AS) * w + S_k
                SHT1 = _ap(STH, k_exact * NC2, [[1, tail * NC2]])
                SHT2 = _ap(STH, k_exact * NC2, [[NC2, tail], [1, NC2]])
                nc.vector.tensor_mul(out=SHT2, in0=cof2(3), in1=ub)
                nc.vector.tensor_add(out=SHT1, in0=SHT1, in1=cof(2))
                nc.vector.tensor_mul(out=SHT2, in0=SHT2, in1=wb)
                nc.vector.tensor_add(out=SHT2, in0=SHT2, in1=s20b)

            # fp16 convert of the head cumsum (slots 0..k_exact-1)
            nc.vector.tensor_copy(out=_ap(STH, 0, [[1, k_exact * NC2]]),
                                  in_=_ap(ST, 0, [[1, k_exact * NC2]]))

            # ---------------- trig ----------------
            SINV = pool.tile([PB, T * NC2], F16)
            COSV = pool.tile([PB, T * NC2], F16)
            ang = _ap(VT, NC2, [[1, k_exact * NC2]])
            nc.scalar.activation(out=_ap(SINV, 0, [[1, k_exact * NC2]]),
                                 in_=ang, func=ACT.Sin)
            nc.scalar.activation(out=ang, in_=ang, func=ACT.Abs)
            nc.scalar.activation(out=_ap(COSV, 0, [[1, k_exact * NC2]]),
                                 in_=ang, func=ACT.Sin, bias=halfpi[:],
                                 scale=-1.0)
            if tail > 0:
                VTT1 = _ap(VTT, 0, [[1, tail * NC2]])
                nc.scalar.activation(out=_ap(SINV, k_exact * NC2,
                                             [[1, tail * NC2]]),
                                     in_=VTT1, func=ACT.Sin)
                nc.scalar.activation(out=_ap(COSV, k_exact * NC2,
                                             [[1, tail * NC2]]),
                                     in_=VTT1, func=ACT.Sin, bias=halfpi[:],
                                     scale=-1.0)

            # ---------------- distance phase (fp16, t-major) -----------
            def half(tl, off):
                return _ap(tl, off, [[NC2, T], [1, A]])

            SEh, SAh = half(STH, 0), half(STH, A)
            SINE, SINA = half(SINV, 0), half(SINV, A)
            COSE, COSA = half(COSV, 0), half(COSV, A)

            PXY = pool.tile([PB, 2 * NT], F16, tag="tPXY2")
            SCR = pool.tile([PB, 2 * NT], F16, tag="tSCR2")
            R12 = pool.tile([PB, 2 * NT], F16)
            R34 = pool.tile([PB, 2 * NT], F16)

            def v2(tl, off):
                return _ap(tl, off, [[A, T], [1, A]])

            def v1(tl, off):
                return _ap(tl, off, [[1, NT]])

            PX2, PY2 = v2(PXY, 0), v2(PXY, NT)
            S1_2, S2_2 = v2(SCR, 0), v2(SCR, NT)
            S1_1, S2_1 = v1(SCR, 0), v1(SCR, NT)
            R1X2, R1Y2 = v2(R12, 0), v2(R12, NT)
            R2X2, R2Y2 = v2(R34, 0), v2(R34, NT)
            R1X1, R1Y1 = v1(R12, 0), v1(R12, NT)
            R2X1, R2Y1 = v1(R34, 0), v1(R34, NT)

            nc.vector.tensor_mul(out=S1_2, in0=SAh, in1=cbh(C_CADT))
            nc.vector.tensor_add(out=S1_2, in0=S1_2, in1=cbh(C_P0X))
            nc.vector.tensor_mul(out=S2_2, in0=SEh, in1=cbh(C_CEDT))
            nc.vector.tensor_sub(out=PX2, in0=S1_2, in1=S2_2)
            nc.vector.tensor_mul(out=S1_2, in0=SAh, in1=cbh(C_SADT))
            nc.vector.tensor_add(out=S1_2, in0=S1_2, in1=cbh(C_P0Y))
            nc.vector.tensor_mul(out=S2_2, in0=SEh, in1=cbh(C_SEDT))
            nc.vector.tensor_sub(out=PY2, in0=S1_2, in1=S2_2)

            nc.vector.tensor_mul(out=R1X2, in0=SINE, in1=PY2)
            nc.vector.tensor_mul(out=R1Y2, in0=SINE, in1=PX2)
            nc.vector.tensor_mul(out=R2X2, in0=SINA, in1=PY2)
            nc.vector.tensor_mul(out=R2Y2, in0=SINA, in1=PX2)
            nc.vector.tensor_mul(out=S1_2, in0=COSE, in1=PX2)
            nc.vector.tensor_add(out=R1X1, in0=R1X1, in1=S1_1)
            nc.vector.tensor_mul(out=S2_2, in0=COSE, in1=PY2)
            nc.vector.tensor_sub(out=R1Y1, in0=S2_1, in1=R1Y1)
            nc.vector.tensor_mul(out=S1_2, in0=COSA, in1=PX2)
            nc.vector.tensor_add(out=R2X1, in0=R2X1, in1=S1_1)
            nc.vector.tensor_mul(out=S2_2, in0=COSA, in1=PY2)
            nc.vector.tensor_sub(out=R2Y1, in0=R2Y1, in1=S2_1)

            for R in (R1X1, R1Y1, R2X1, R2Y1):
                nc.scalar.activation(out=R, in_=R, func=ACT.Abs)
            nc.vector.tensor_add(out=R1X2, in0=R1X2, in1=cbh(C_D1))
            nc.vector.tensor_tensor(out=R1X1, in0=R1X1, in1=R1Y1, op=OP.max)
            nc.vector.tensor_add(out=R2X2, in0=R2X2, in1=cbh(C_D2))
            nc.vector.tensor_tensor(out=R2X1, in0=R2X1, in1=R2Y1, op=OP.max)
            nc.vector.tensor_add(out=R1X2, in0=R1X2, in1=cbh(C_D3))
            nc.vector.tensor_tensor(out=R1X1, in0=R1X1, in1=R2X1, op=OP.max)

            W = SCR
            nc.vector.tensor_tensor(out=_ap(W, 0, [[1, 25 * A]]),
                                    in0=_ap(R12, 0, [[1, 25 * A]]),
                                    in1=_ap(R12, 25 * A, [[1, 25 * A]]),
                                    op=OP.min)
            n = 25
            while n > 1:
                h = n // 2
                if n % 2:
                    nc.vector.tensor_tensor(
                        out=_ap(W, 0, [[1, A]]), in0=_ap(W, 0, [[1, A]]),
                        in1=_ap(W, (n - 1) * A, [[1, A]]), op=OP.min)
                nc.vector.tensor_tensor(out=_ap(W, 0, [[1, h * A]]),
                                        in0=_ap(W, 0, [[1, h * A]]),
                                        in1=_ap(W, h * A, [[1, h * A]]),
                                        op=OP.min)
                n = h

            H = pool.tile([PB, A], F32)
            nc.vector.tensor_copy(out=H[:], in_=_ap(W, 0, [[1, A]]))
            nc.vector.tensor_sub(out=H[:], in0=H[:], in1=c(C_K2Y))
            OUTT = pool.tile([PB, A], F32)
            nc.scalar.activation(out=H[:], in_=H[:], func=ACT.Tanh, scale=0.1)
            nc.vector.tensor_scalar_mul(out=OUTT[:], in0=H[:], scalar1=5.0)
            nc.sync.dma_start(out=out[:], in_=OUTT[:])

    nc.compile()
    return nc


def _get_nc(dt_uniform, k_red, k_exact):
    key = ("nc", dt_uniform, k_red, k_exact)
    if key not in _cache:
        _cache[key] = _build(dt_uniform, k_red, k_exact)
    return _cache[key]


def _fit_tail_coefs(dt, k_exact, tail):
    """Per-slot deg-1 fits in u=w^2 of v_{k+m} = w*FV_m(u) and
    (S_{k+m} - S_k) = w*FS_m(u), m=0..tail-1 (FS_0 = 0, FV row m uses
    phi_{m+1}). Returns fp16 [4, tail, NC2]-expanded array."""
    w = np.linspace(-0.21, 0.21, 20001)
    w = w[np.abs(w) > 1e-9]
    u = w * w
    basis = np.stack([np.ones_like(u), u], axis=1)
    x = w.copy()
    AV = np.zeros(tail); BV = np.zeros(tail)
    AS = np.zeros(tail); BS = np.zeros(tail)
    pref = np.zeros_like(w)
    for m in range(tail):
        x = x - 9.0 * dt * np.tanh(2.0 * x)   # phi_{m+1}
        cv, *_ = np.linalg.lstsq(basis, x / w, rcond=None)
        AV[m], BV[m] = cv
        if m > 0:
            cs, *_ = np.linalg.lstsq(basis, pref / w, rcond=None)
            AS[m], BS[m] = cs
        pref = pref + x
    co = np.stack([AV, BV, AS, BS])             # [4, tail]
    co = np.repeat(co[:, :, None], NC2, axis=2)  # [4, tail, NC2]
    return np.ascontiguousarray(co.reshape(1, -1).astype(np.float16))


def _params_for(data: np.ndarray):
    dt = data[..., 14]
    dt0 = float(dt.flat[0])
    dt_uniform = dt0 if bool(np.all(dt == dt0)) else None
    vmax = float(np.abs(data[..., [2, 6]]).max())
    dt_min = float(dt.min())
    dt_max = float(dt.max())
    shrink = 9.0 * dt_min * 0.9997
    if 9.0 * dt_max > np.pi or shrink <= 1e-6:
        k_red = T
    else:
        k_red = int(min(T, max(0, np.ceil((vmax - np.pi) / shrink) + 1)))
    # k_exact: steps (verified on a dense grid) until |v| <= 0.2
    if dt_uniform is None:
        k_exact = T
    else:
        g = np.linspace(0.0, vmax + 1e-3, 200001)
        k_exact = T
        for j in range(1, T + 1):
            g = g - 9.0 * dt_uniform * np.tanh(2.0 * g)
            if np.abs(g).max() <= 0.2:
                k_exact = j
                break
    k_exact = int(min(T, max(k_exact, k_red, 1)))
    return dt_uniform, k_red, k_exact


def _host_inputs(data, dt_uniform, k_exact):
    """data [B, A, F] -> field-major data2 [B, F*A], dv [B, 2A], coef."""
    d2 = np.ascontiguousarray(data.transpose(0, 2, 1)).reshape(B, F * A)
    dv = np.ascontiguousarray(data[..., [2, 6]].transpose(0, 2, 1)
                              ).reshape(B, 2 * A)
    tail = T - k_exact
    co = None
    if tail > 0:
        key = ("coef", dt_uniform, k_exact)
        if key not in _cache:
            _cache[key] = _fit_tail_coefs(dt_uniform, k_exact, tail)
        co = np.ascontiguousarray(
            np.broadcast_to(_cache[key], (PB, 4 * tail * NC2)))
    return d2, dv, co


def _in_maps_for(data, dt_uniform, k_red, k_exact):
    d2, dv, co = _host_inputs(data, dt_uniform, k_exact)
    in_maps = []
    for c in range(N_CORES):
        m = {"data2": d2[c * PB:(c + 1) * PB], "dv": dv[c * PB:(c + 1) * PB]}
        if co is not None:
            m["coef"] = co
        in_maps.append(m)
    return in_maps


def _make_runner(nc):
    import jax
    from jax.sharding import Mesh, PartitionSpec
    from jax.experimental.shard_map import shard_map
    from concourse import bass2jax, mybir as _mybir

    bass2jax.install_neuronx_cc_hook()
    partition_name = (nc.partition_id_tensor.name
                      if nc.partition_id_tensor else None)
    in_names, out_names, out_avals, zero_outs = [], [], [], []
    for alloc in nc.m.functions[0].allocations:
        if not isinstance(alloc, _mybir.MemoryLocationSet):
            continue
        name = alloc.memorylocations[0].name
        if alloc.kind == "ExternalInput":
            if name != partition_name:
                in_names.append(name)
        elif alloc.kind == "ExternalOutput":
            shape = tuple(alloc.tensor_shape)
            dtype = _mybir.dt.np(alloc.dtype)
            out_names.append(name)
            out_avals.append(jax.core.ShapedArray(shape, dtype))
            zero_outs.append(np.zeros(shape, dtype))
    n_params = len(in_names)
    all_names = in_names + out_names
    if partition_name is not None:
        all_names = all_names + [partition_name]
    donate = tuple(range(n_params, n_params + len(out_names)))

    def _body(*args):
        operands = list(args)
        if partition_name is not None:
            operands.append(bass2jax.partition_id_tensor())
        outs = bass2jax._bass_exec_p.bind(
            *operands, out_avals=tuple(out_avals), in_names=tuple(all_names),
            out_names=tuple(out_names), lowering_input_output_aliases=(),
            sim_require_finite=True, sim_require_nnan=True, nc=nc)
        return tuple(outs)

    mesh = Mesh(np.asarray(jax.devices()[:N_CORES]), ("core",))
    in_specs = (PartitionSpec("core"),) * (n_params + len(out_names))
    out_specs = (PartitionSpec("core"),) * len(out_names)
    sharded = jax.jit(
        shard_map(_body, mesh=mesh, in_specs=in_specs, out_specs=out_specs,
                  check_rep=False),
        donate_argnums=donate, keep_unused=True)
    concat_zeros = [np.zeros((N_CORES * z.shape[0], *z.shape[1:]), z.dtype)
                    for z in zero_outs]

    def run(named_inputs):  # dict name -> [B-like, ...] concatenated arrays
        args = [named_inputs[n] for n in in_names]
        outs = sharded(*args, *[z.copy() for z in concat_zeros])
        return np.asarray(outs[out_names.index("out")])

    return run


def _run(data: np.ndarray, trace: bool = False):
    data = np.ascontiguousarray(data, dtype=np.float32)
    assert data.shape == (B, A, F), data.shape
    dt_uniform, k_red, k_exact = _params_for(data)
    nc = _get_nc(dt_uniform, k_red, k_exact)
    in_maps = _in_maps_for(data, dt_uniform, k_red, k_exact)
    res = run_bass_kernel_spmd(nc, in_maps, core_ids=list(range(N_CORES)),
                               trace=trace)
    full = np.concatenate([res.results[c]["out"] for c in range(N_CORES)],
                          axis=0)
    return full, res


def kernel(data: np.ndarray) -> np.ndarray:
    data = np.ascontiguousarray(data, dtype=np.float32)
    assert data.shape == (B, A, F), data.shape
    dt_uniform, k_red, k_exact = _params_for(data)
    key = ("runner", dt_uniform, k_red, k_exact)
    if key not in _cache:
        _cache[key] = _make_runner(_get_nc(dt_uniform, k_red, k_exact))
    d2, dv, co = _host_inputs(data, dt_uniform, k_exact)
    named = {"data2": d2, "dv": dv}
    if co is not None:
        named["coef"] = np.ascontiguousarray(
            np.broadcast_to(co[None, :, :], (N_CORES, PB, co.shape[1]))
        ).reshape(N_CORES * PB, -1)
    return _cache[key](named).astype(np.float32)


# revision 3
# speedup vs baseline: 1.0703x; 1.0005x over previous
"""Trainium2 Bass kernel for nn_BackupBarrierCBF — v3.

v2 (121.8us) + rollout restructure:
  - Serial ACT-Tanh chain only until |v| provably <= 0.2 (k_exact ~ 18
    steps from vmax~15; host-verified on a dense grid). The remaining
    TAIL = 50-k_exact slots are VECTORIZED: v_{k+m} = w*FV_m(w^2) and
    S_{k+m} = S_k + w*FS_m(w^2) with per-slot deg-1 polynomial fits in
    u = w^2 computed on host (max fit err ~1.3e-3 on v, 7e-4 on S —
    fp16-noise level). Coefficients ship pre-broadcast as an extra
    fp16 DMA input (hidden behind the serial phase).
  - Tail angles |v|<=0.2: no range reduction, no abs pass for cos.
  - Constants phase: single batched range-reduce + one Sin over [4A]
    (shift-fold trick for the cos halves), one Sqrt over [2A]; all DVE
    prep interleaved into the chain's DVE gaps; ACT ops after the chain
    (no Sin/Tanh table thrash).
  - STH convert on DVE post-chain (off the ACT trig path).
Distance phase identical to v2 (fp16 t-major, 2x DVE).
"""
import numpy as np
import concourse.bass as bass
import concourse.bacc as bacc
import concourse.tile as tile
from concourse import mybir
from concourse.bass_utils import run_bass_kernel_spmd

F32 = mybir.dt.float32
F16 = mybir.dt.float16
I32 = mybir.dt.int32
OP = mybir.AluOpType
ACT = mybir.ActivationFunctionType

B, A, F = 1024, 64, 15
N_CORES = 8
PB = B // N_CORES
T = 50
NC2 = 2 * A
NT = T * A
TWO_PI = float(2.0 * np.pi)
MAGIC = float(1.5 * 2 ** 23)   # fp32 round-to-nearest-even bias

_cache: dict = {}


def _ap(t: bass.AP, extra_offset: int, free_dims: list) -> bass.AP:
    return bass.AP(tensor=t.tensor, offset=t.offset + extra_offset,
                   ap=[list(t.ap[0])] + [list(d) for d in free_dims])


def _build(dt_uniform, k_red, k_exact):
    tail = T - k_exact
    m0 = min(15, tail)          # deg-1 V slots; V deg-0 beyond, S deg-0 all
    ncoef = (m0 + 2 * tail) * NC2   # [AV1(m0) | BV1(m0) | AV0(tail-m0) | AS0(tail)]
    nc = bacc.Bacc("TRN2", target_bir_lowering=False)
    # field-major input [PB, F*A]: field k occupies cols [k*A, (k+1)*A)
    data = nc.dram_tensor("data2", [PB, F * A], F32, kind="ExternalInput")
    # duplicate of the two v columns; tiny DMA so the chain starts early
    dvin = nc.dram_tensor("dv", [PB, 2 * A], F32, kind="ExternalInput")
    coef = (nc.dram_tensor("coef", [PB, ncoef], F16,
                           kind="ExternalInput") if tail > 0 else None)
    out = nc.dram_tensor("out", [PB, A], F32, kind="ExternalOutput")

    with tile.TileContext(nc) as tc:
        with tc.tile_pool(name="pool", bufs=1) as pool:
            Dv = pool.tile([PB, 2 * A], F32)
            nc.sync.dma_start(out=Dv[:], in_=dvin[:])
            D = pool.tile([PB, F * A], F32)
            nc.sync.dma_start(out=D[:], in_=data[:])
            CO = None
            if tail > 0:
                CO = pool.tile([PB, ncoef], F16)
                nc.sync.dma_start(out=CO[:], in_=coef[:])

            def fld(k):
                return _ap(D, k * A, [[1, A]])

            halfpi = pool.tile([PB, 1], F32)
            cons = pool.tile([PB, 12, A], F32)
            consh = pool.tile([PB, 12, A], F16)
            scr = pool.tile([PB, 10, A], F32)
            ki = pool.tile([PB, 4, A], I32)

            def c(i):
                return _ap(cons, i * A, [[1, A]])

            def cbh(i):
                return _ap(consh, i * A, [[0, T], [1, A]])

            def s(i):
                return _ap(scr, i * A, [[1, A]])

            C_P0X, C_P0Y = 0, 1
            C_D1, C_D2, C_D3, C_K2Y = 2, 3, 4, 5
            C_CEDT, C_SEDT, C_CADT, C_SADT = 6, 7, 8, 9
            C_RE, C_RA = 10, 11

            # ---- constants prep (pure DVE), interleaved into the chain ----
            # Batched sincos args: scr rows 0..3 hold args for
            # [cos(th_e), sin(th_e), cos(th_a), sin(th_a)] -> cons 6..9.
            prep = []
            S4 = _ap(scr, 0, [[1, 4 * A]])
            KI4 = _ap(ki, 0, [[1, 4 * A]])
            prep.append(lambda: nc.vector.memset(halfpi[:], float(np.pi / 2)))
            prep.append(lambda: nc.vector.tensor_copy(
                out=_ap(scr, 0, [[A, 2], [1, A]]),
                in_=_ap(D, 3 * A, [[0, 2], [1, A]])))
            prep.append(lambda: nc.vector.tensor_copy(
                out=_ap(scr, 2 * A, [[A, 2], [1, A]]),
                in_=_ap(D, 7 * A, [[0, 2], [1, A]])))
            # sc = th/2pi (+0.25 on the cos rows 0 and 2), then magic-number
            # round-to-nearest (adding 1.5*2^23 snaps the mantissa to ulp=1)
            prep.append(lambda: nc.vector.tensor_scalar(
                out=S4, in0=S4, scalar1=1.0 / TWO_PI, scalar2=0.0,
                op0=OP.mult, op1=OP.add))
            prep.append(lambda: nc.vector.tensor_scalar(
                out=_ap(scr, 0, [[2 * A, 2], [1, A]]),
                in0=_ap(scr, 0, [[2 * A, 2], [1, A]]),
                scalar1=1.0, scalar2=0.25, op0=OP.mult, op1=OP.add))
            prep.append(lambda: nc.vector.tensor_scalar(
                out=S4, in0=S4, scalar1=MAGIC, scalar2=MAGIC,
                op0=OP.add, op1=OP.subtract))
            # subtract the shift back on cos rows: k - 0.25
            prep.append(lambda: nc.vector.tensor_scalar(
                out=_ap(scr, 0, [[2 * A, 2], [1, A]]),
                in0=_ap(scr, 0, [[2 * A, 2], [1, A]]),
                scalar1=1.0, scalar2=-0.25, op0=OP.mult, op1=OP.add))
            # arg = th - 2pi*k  (th re-read strided)
            prep.append(lambda: nc.vector.scalar_tensor_tensor(
                out=_ap(scr, 0, [[1, A]]), in0=_ap(scr, 0, [[1, A]]),
                scalar=-TWO_PI, in1=fld(3), op0=OP.mult, op1=OP.add))
            prep.append(lambda: nc.vector.scalar_tensor_tensor(
                out=_ap(scr, A, [[1, A]]), in0=_ap(scr, A, [[1, A]]),
                scalar=-TWO_PI, in1=fld(3), op0=OP.mult, op1=OP.add))
            prep.append(lambda: nc.vector.scalar_tensor_tensor(
                out=_ap(scr, 2 * A, [[1, A]]), in0=_ap(scr, 2 * A, [[1, A]]),
                scalar=-TWO_PI, in1=fld(7), op0=OP.mult, op1=OP.add))
            prep.append(lambda: nc.vector.scalar_tensor_tensor(
                out=_ap(scr, 3 * A, [[1, A]]), in0=_ap(scr, 3 * A, [[1, A]]),
                scalar=-TWO_PI, in1=fld(7), op0=OP.mult, op1=OP.add))
            # sumsq for r_e, r_a into scr rows 4 (ego) and 5 (agent)
            prep.append(lambda: nc.vector.tensor_mul(
                out=s(4), in0=fld(8), in1=fld(8)))
            prep.append(lambda: nc.vector.tensor_mul(
                out=s(6), in0=fld(9), in1=fld(9)))
            prep.append(lambda: nc.vector.tensor_add(
                out=s(4), in0=s(4), in1=s(6)))
            prep.append(lambda: nc.vector.tensor_mul(
                out=s(5), in0=fld(11), in1=fld(11)))
            prep.append(lambda: nc.vector.tensor_mul(
                out=s(6), in0=fld(12), in1=fld(12)))
            prep.append(lambda: nc.vector.tensor_add(
                out=s(5), in0=s(5), in1=s(6)))
            # d1/d2/p0
            prep.append(lambda: nc.vector.tensor_sub(
                out=s(6), in0=fld(9), in1=fld(8)))
            prep.append(lambda: nc.vector.tensor_scalar_mul(
                out=c(C_D1), in0=s(6), scalar1=0.5))
            prep.append(lambda: nc.vector.tensor_sub(
                out=s(7), in0=fld(12), in1=fld(11)))
            prep.append(lambda: nc.vector.tensor_scalar_mul(
                out=c(C_D2), in0=s(7), scalar1=0.5))
            prep.append(lambda: nc.vector.tensor_sub(
                out=c(C_P0X), in0=fld(4), in1=fld(0)))
            prep.append(lambda: nc.vector.tensor_sub(
                out=c(C_P0Y), in0=fld(5), in1=fld(1)))

            # ---------------- rollout: serial head ----------------
            VT = pool.tile([PB, (k_exact + 1) * NC2], F32)
            ST = pool.tile([PB, (k_exact + 1) * NC2], F32)
            STH = pool.tile([PB, T * NC2], F16)
            G = pool.tile([PB, NC2], F32)

            def vslot(j):
                if j == 0:
                    return _ap(Dv, 0, [[A, 2], [1, A]])
                return _ap(VT, j * NC2, [[1, NC2]])

            def stslot(k):
                return _ap(ST, k * NC2, [[1, NC2]])

            nc.vector.tensor_copy(out=stslot(0), in_=vslot(0))

            NDT2 = None
            if dt_uniform is None:
                NDT2 = pool.tile([PB, NC2], F32)
                nc.vector.tensor_scalar_mul(
                    out=NDT2[:], in0=_ap(D, 14 * A, [[0, 2], [1, A]]),
                    scalar1=-9.0)

            # Interleave preps into the chain from step 4 on (2 per step):
            # by then the big D DMA has landed, so a prep never stalls the
            # chain's STTs (emitting preps after the loop would let the
            # scheduler place all of them — waiting on D — ahead of STT1).
            pi = 0
            for j in range(1, k_exact + 1):
                nc.scalar.activation(out=G[:], in_=vslot(j - 1),
                                     func=ACT.Tanh, scale=2.0)
                if dt_uniform is None:
                    nc.vector.tensor_mul(out=G[:], in0=G[:], in1=NDT2[:])
                    nc.vector.tensor_add(out=vslot(j), in0=vslot(j - 1),
                                         in1=G[:])
                else:
                    nc.vector.scalar_tensor_tensor(
                        out=vslot(j), in0=G[:], scalar=-9.0 * float(dt_uniform),
                        in1=vslot(j - 1), op0=OP.mult, op1=OP.add)
                nc.vector.tensor_add(out=stslot(j), in0=stslot(j - 1),
                                     in1=vslot(j))
                if j >= 4:
                    for _ in range(2):
                        if pi < len(prep):
                            prep[pi]()
                            pi += 1
            while pi < len(prep):
                prep[pi]()
                pi += 1
            # chain-end gate: z = 0 * v_k, then += 0 on the ACT-const
            # inputs so Sqrt/Sin (and their table loads) cannot be
            # scheduled into the middle of the Tanh chain.
            zg = pool.tile([PB, 1], F32)
            nc.vector.tensor_scalar_mul(out=zg[:],
                                        in0=_ap(VT, k_exact * NC2, [[1, 1]]),
                                        scalar1=0.0)
            nc.vector.tensor_add(out=_ap(scr, 0, [[NC2, 2], [1, NC2]]),
                                 in0=_ap(scr, 0, [[NC2, 2], [1, NC2]]),
                                 in1=_ap(zg, 0, [[0, 2], [0, NC2]]))
            nc.vector.tensor_add(out=_ap(scr, 4 * A, [[A, 2], [1, A]]),
                                 in0=_ap(scr, 4 * A, [[A, 2], [1, A]]),
                                 in1=_ap(zg, 0, [[0, 2], [0, A]]))

            # ---- range-reduce head angle slots 1..k_red (in place) ----
            # magic-number RNE round: k = (x/2pi + M) - M, both ts ops 2x
            if k_red > 0:
                MS = pool.tile([PB, NC2 * k_red], F32, tag="tPXY")
                red_view = _ap(VT, NC2, [[1, NC2 * k_red]])
                nc.vector.tensor_scalar(out=MS[:], in0=red_view,
                                        scalar1=1.0 / TWO_PI, scalar2=MAGIC,
                                        op0=OP.mult, op1=OP.add)
                nc.vector.tensor_scalar(out=MS[:], in0=MS[:],
                                        scalar1=MAGIC, scalar2=-TWO_PI,
                                        op0=OP.subtract, op1=OP.mult)
                nc.vector.tensor_add(out=red_view, in0=red_view, in1=MS[:])

            # ---- ACT constants: one Sqrt over [2A], one Sin over [4A] ----
            nc.scalar.activation(out=_ap(cons, C_RE * A, [[1, 2 * A]]),
                                 in_=_ap(scr, 4 * A, [[1, 2 * A]]),
                                 func=ACT.Sqrt, scale=0.25)
            # gate the Sin batch on the Sqrt output so the scheduler can't
            # interleave Sqrt between the Sin-table trig passes
            zg2 = pool.tile([PB, 1], F32)
            nc.vector.tensor_scalar_mul(out=zg2[:],
                                        in0=_ap(cons, C_RE * A, [[1, 1]]),
                                        scalar1=0.0)
            nc.vector.tensor_add(out=_ap(scr, 0, [[NC2, 2], [1, NC2]]),
                                 in0=_ap(scr, 0, [[NC2, 2], [1, NC2]]),
                                 in1=_ap(zg2, 0, [[0, 2], [0, NC2]]))
            nc.scalar.activation(out=_ap(cons, C_CEDT * A, [[1, 4 * A]]),
                                 in_=S4, func=ACT.Sin)
            # finish constants on DVE (cheap, off chain)
            for i in (C_CADT, C_SADT, C_CEDT, C_SEDT):
                nc.vector.tensor_mul(out=c(i), in0=c(i), in1=fld(14))
            nc.vector.scalar_tensor_tensor(
                out=s(8), in0=fld(9), scalar=0.5, in1=c(C_RA),
                op0=OP.mult, op1=OP.add)          # k1y
            nc.vector.scalar_tensor_tensor(
                out=c(C_K2Y), in0=fld(12), scalar=0.5, in1=c(C_RE),
                op0=OP.mult, op1=OP.add)          # k2y
            nc.vector.tensor_sub(out=c(C_D3), in0=c(C_K2Y), in1=s(8))
            nc.vector.tensor_copy(out=_ap(consh, 0, [[1, 12 * A]]),
                                  in_=_ap(cons, 0, [[1, 12 * A]]))

            # ---------------- vectorized tail ----------------
            VTT = pool.tile([PB, max(1, tail) * NC2], F16)
            wv = pool.tile([PB, NC2], F16)
            uv = pool.tile([PB, NC2], F16)
            s20h = pool.tile([PB, NC2], F16)
            if tail > 0:
                nc.vector.tensor_copy(out=wv[:], in_=vslot(k_exact))
                nc.vector.tensor_mul(out=uv[:], in0=vslot(k_exact),
                                     in1=vslot(k_exact))
                nc.vector.tensor_copy(out=s20h[:], in_=stslot(k_exact))

                def bcast(tl, n):
                    return _ap(tl, 0, [[0, n], [1, NC2]])

                # angles, deg-1 slots 0..m0: VTT = (BV1*u + AV1) * w
                O_BV1, O_AV0, O_AS0 = m0 * NC2, 2 * m0 * NC2, \
                    (m0 + tail) * NC2
                V1a = _ap(VTT, 0, [[1, m0 * NC2]])
                V1b = _ap(VTT, 0, [[NC2, m0], [1, NC2]])
                nc.vector.tensor_mul(out=V1b,
                                     in0=_ap(CO, O_BV1,
                                             [[NC2, m0], [1, NC2]]),
                                     in1=bcast(uv, m0))
                nc.vector.tensor_add(out=V1a, in0=V1a,
                                     in1=_ap(CO, 0, [[1, m0 * NC2]]))
                nc.vector.tensor_mul(out=V1b, in0=V1b, in1=bcast(wv, m0))
                # angles, deg-0 slots m0..tail: VTT = AV0 * w
                if tail > m0:
                    nc.vector.tensor_mul(
                        out=_ap(VTT, m0 * NC2, [[NC2, tail - m0], [1, NC2]]),
                        in0=_ap(CO, O_AV0, [[NC2, tail - m0], [1, NC2]]),
                        in1=bcast(wv, tail - m0))
                # cumsum, deg-0 all slots: STH[k_exact..] = AS0 * w + S_k
                SHT2 = _ap(STH, k_exact * NC2, [[NC2, tail], [1, NC2]])
                nc.vector.tensor_mul(out=SHT2,
                                     in0=_ap(CO, O_AS0,
                                             [[NC2, tail], [1, NC2]]),
                                     in1=bcast(wv, tail))
                nc.vector.tensor_add(out=SHT2, in0=SHT2, in1=bcast(s20h, tail))

            # fp16 convert of the head cumsum (slots 0..k_exact-1) on ACT
            # (it has slack here; DVE is the critical path)
            nc.scalar.activation(out=_ap(STH, 0, [[1, k_exact * NC2]]),
                                 in_=_ap(ST, 0, [[1, k_exact * NC2]]),
                                 func=ACT.Copy)

            # ---------------- trig ----------------
            SINV = pool.tile([PB, T * NC2], F16)
            COSV = pool.tile([PB, T * NC2], F16)
            ang = _ap(VT, NC2, [[1, k_exact * NC2]])
            nc.scalar.activation(out=_ap(SINV, 0, [[1, k_exact * NC2]]),
                                 in_=ang, func=ACT.Sin)
            nc.scalar.activation(out=ang, in_=ang, func=ACT.Abs)
            nc.scalar.activation(out=_ap(COSV, 0, [[1, k_exact * NC2]]),
                                 in_=ang, func=ACT.Sin, bias=halfpi[:],
                                 scale=-1.0)
            if tail > 0:
                VTT1 = _ap(VTT, 0, [[1, tail * NC2]])
                nc.scalar.activation(out=_ap(SINV, k_exact * NC2,
                                             [[1, tail * NC2]]),
                                     in_=VTT1, func=ACT.Sin)
                nc.scalar.activation(out=_ap(COSV, k_exact * NC2,
                                             [[1, tail * NC2]]),
                                     in_=VTT1, func=ACT.Sin, bias=halfpi[:],
                                     scale=-1.0)

            # ---------------- distance phase (fp16, t-major) -----------
            def half(tl, off):
                return _ap(tl, off, [[NC2, T], [1, A]])

            SEh, SAh = half(STH, 0), half(STH, A)
            SINE, SINA = half(SINV, 0), half(SINV, A)
            COSE, COSA = half(COSV, 0), half(COSV, A)

            PXY = pool.tile([PB, 2 * NT], F16, tag="tPXY2")
            SCR = pool.tile([PB, 2 * NT], F16, tag="tSCR2")
            R12 = pool.tile([PB, 2 * NT], F16)
            R34 = pool.tile([PB, 2 * NT], F16)

            def v2(tl, off):
                return _ap(tl, off, [[A, T], [1, A]])

            def v1(tl, off):
                return _ap(tl, off, [[1, NT]])

            PX2, PY2 = v2(PXY, 0), v2(PXY, NT)
            S1_2, S2_2 = v2(SCR, 0), v2(SCR, NT)
            S1_1, S2_1 = v1(SCR, 0), v1(SCR, NT)
            R1X2, R1Y2 = v2(R12, 0), v2(R12, NT)
            R2X2, R2Y2 = v2(R34, 0), v2(R34, NT)
            R1X1, R1Y1 = v1(R12, 0), v1(R12, NT)
            R2X1, R2Y1 = v1(R34, 0), v1(R34, NT)

            nc.vector.tensor_mul(out=S1_2, in0=SAh, in1=cbh(C_CADT))
            nc.vector.tensor_add(out=S1_2, in0=S1_2, in1=cbh(C_P0X))
            nc.vector.tensor_mul(out=S2_2, in0=SEh, in1=cbh(C_CEDT))
            nc.vector.tensor_sub(out=PX2, in0=S1_2, in1=S2_2)
            nc.vector.tensor_mul(out=S1_2, in0=SAh, in1=cbh(C_SADT))
            nc.vector.tensor_add(out=S1_2, in0=S1_2, in1=cbh(C_P0Y))
            nc.vector.tensor_mul(out=S2_2, in0=SEh, in1=cbh(C_SEDT))
            nc.vector.tensor_sub(out=PY2, in0=S1_2, in1=S2_2)

            nc.vector.tensor_mul(out=R1X2, in0=SINE, in1=PY2)
            nc.vector.tensor_mul(out=R1Y2, in0=SINE, in1=PX2)
            nc.vector.tensor_mul(out=R2X2, in0=SINA, in1=PY2)
            nc.vector.tensor_mul(out=R2Y2, in0=SINA, in1=PX2)
            nc.vector.tensor_mul(out=S1_2, in0=COSE, in1=PX2)
            nc.vector.tensor_add(out=R1X1, in0=R1X1, in1=S1_1)
            nc.vector.tensor_mul(out=S2_2, in0=COSE, in1=PY2)
            nc.vector.tensor_sub(out=R1Y1, in0=S2_1, in1=R1Y1)
            nc.vector.tensor_mul(out=S1_2, in0=COSA, in1=PX2)
            nc.vector.tensor_add(out=R2X1, in0=R2X1, in1=S1_1)
            nc.vector.tensor_mul(out=S2_2, in0=COSA, in1=PY2)
            nc.vector.tensor_sub(out=R2Y1, in0=R2Y1, in1=S2_1)

            for R in (R1X1, R1Y1, R2X1, R2Y1):
                nc.scalar.activation(out=R, in_=R, func=ACT.Abs)
            nc.vector.tensor_add(out=R1X2, in0=R1X2, in1=cbh(C_D1))
            nc.vector.tensor_tensor(out=R1X1, in0=R1X1, in1=R1Y1, op=OP.max)
            nc.vector.tensor_add(out=R2X2, in0=R2X2, in1=cbh(C_D2))
            nc.vector.tensor_tensor(out=R2X1, in0=R2X1, in1=R2Y1, op=OP.max)
            nc.vector.tensor_add(out=R1X2, in0=R1X2, in1=cbh(C_D3))
            nc.vector.tensor_tensor(out=R1X1, in0=R1X1, in1=R2X1, op=OP.max)

            W = SCR
            nc.vector.tensor_tensor(out=_ap(W, 0, [[1, 25 * A]]),
                                    in0=_ap(R12, 0, [[1, 25 * A]]),
                                    in1=_ap(R12, 25 * A, [[1, 25 * A]]),
                                    op=OP.min)
            n = 25
            while n > 1:
                h = n // 2
                if n % 2:
                    nc.vector.tensor_tensor(
                        out=_ap(W, 0, [[1, A]]), in0=_ap(W, 0, [[1, A]]),
                        in1=_ap(W, (n - 1) * A, [[1, A]]), op=OP.min)
                nc.vector.tensor_tensor(out=_ap(W, 0, [[1, h * A]]),
                                        in0=_ap(W, 0, [[1, h * A]]),
                                        in1=_ap(W, h * A, [[1, h * A]]),
                                        op=OP.min)
                n = h

            H = pool.tile([PB, A], F32)
            nc.vector.tensor_sub(out=H[:], in0=_ap(W, 0, [[1, A]]),
                                 in1=c(C_K2Y))
            OUTT = pool.tile([PB, A], F32)
            nc.scalar.activation(out=H[:], in_=H[:], func=ACT.Tanh, scale=0.1)
            nc.vector.tensor_scalar_mul(out=OUTT[:], in0=H[:], scalar1=5.0)
            nc.sync.dma_start(out=out[:], in_=OUTT[:])

    nc.compile()
    return nc


def _get_nc(dt_uniform, k_red, k_exact):
    key = ("nc", dt_uniform, k_red, k_exact)
    if key not in _cache:
        _cache[key] = _build(dt_uniform, k_red, k_exact)
    return _cache[key]


def _fit_tail_coefs(dt, k_exact, tail):
    """Tail fits in u=w^2: v_{k+m} = w*(AV1_m + BV1_m*u) for m<m0 and
    w*AV0_m beyond (deg-0 err < 1e-3 there); (S_{k+m}-S_k) = w*AS0_m for
    all m (deg-0 S err ~6e-3 is scaled by cadt~0.1 downstream, so it is
    ~6e-4 in position). Layout [AV1(m0)|BV1(m0)|AV0(tail-m0)|AS0(tail)],
    each row expanded to NC2 fp16 values."""
    m0 = min(15, tail)
    w = np.linspace(-0.21, 0.21, 20001)
    w = w[np.abs(w) > 1e-9]
    u = w * w
    basis1 = np.stack([np.ones_like(u), u], axis=1)
    x = w.copy()
    AV1 = np.zeros(m0); BV1 = np.zeros(m0)
    AV0 = np.zeros(tail - m0); AS0 = np.zeros(tail)
    pref = np.zeros_like(w)
    for m in range(tail):
        x = x - 9.0 * dt * np.tanh(2.0 * x)   # phi_{m+1}
        if m < m0:
            cv, *_ = np.linalg.lstsq(basis1, x / w, rcond=None)
            AV1[m], BV1[m] = cv
        else:
            AV0[m - m0] = np.mean(x / w)
        if m > 0:
            AS0[m] = np.mean(pref / w)
        pref = pref + x
    co = np.concatenate([AV1, BV1, AV0, AS0])    # [m0 + 2*tail]
    co = np.repeat(co[:, None], NC2, axis=1)     # [., NC2]
    return np.ascontiguousarray(co.reshape(1, -1).astype(np.float16))


def _params_for(data: np.ndarray):
    dt = data[..., 14]
    dt0 = float(dt.flat[0])
    dt_uniform = dt0 if bool(np.all(dt == dt0)) else None
    vmax = float(np.abs(data[..., [2, 6]]).max())
    dt_min = float(dt.min())
    dt_max = float(dt.max())
    shrink = 9.0 * dt_min * 0.9997
    if 9.0 * dt_max > np.pi or shrink <= 1e-6:
        k_red = T
    else:
        k_red = int(min(T, max(0, np.ceil((vmax - np.pi) / shrink) + 1)))
    # k_exact: steps (verified on a dense grid) until |v| <= 0.2
    if dt_uniform is None:
        k_exact = T
    else:
        g = np.linspace(0.0, vmax + 1e-3, 200001)
        k_exact = T
        for j in range(1, T + 1):
            g = g - 9.0 * dt_uniform * np.tanh(2.0 * g)
            if np.abs(g).max() <= 0.2:
                k_exact = j
                break
    k_exact = int(min(T, max(k_exact, k_red, 1)))
    return dt_uniform, k_red, k_exact


def _host_inputs(data, dt_uniform, k_exact):
    """data [B, A, F] -> field-major data2 [B, F*A], dv [B, 2A], coef."""
    d2 = np.ascontiguousarray(data.transpose(0, 2, 1)).reshape(B, F * A)
    dv = np.ascontiguousarray(data[..., [2, 6]].transpose(0, 2, 1)
                              ).reshape(B, 2 * A)
    tail = T - k_exact
    co = None
    if tail > 0:
        key = ("coef", dt_uniform, k_exact)
        if key not in _cache:
            _cache[key] = _fit_tail_coefs(dt_uniform, k_exact, tail)
        co = np.ascontiguousarray(
            np.broadcast_to(_cache[key], (PB, _cache[key].shape[1])))
    return d2, dv, co


def _in_maps_for(data, dt_uniform, k_red, k_exact):
    d2, dv, co = _host_inputs(data, dt_uniform, k_exact)
    in_maps = []
    for c in range(N_CORES):
        m = {"data2": d2[c * PB:(c + 1) * PB], "dv": dv[c * PB:(c + 1) * PB]}
        if co is not None:
            m["coef"] = co
        in_maps.append(m)
    return in_maps


def _make_runner(nc):
    import jax
    from jax.sharding import Mesh, PartitionSpec
    from jax.experimental.shard_map import shard_map
    from concourse import bass2jax, mybir as _mybir

    bass2jax.install_neuronx_cc_hook()
    partition_name = (nc.partition_id_tensor.name
                      if nc.partition_id_tensor else None)
    in_names, out_names, out_avals, zero_outs = [], [], [], []
    for alloc in nc.m.functions[0].allocations:
        if not isinstance(alloc, _mybir.MemoryLocationSet):
            continue
        name = alloc.memorylocations[0].name
        if alloc.kind == "ExternalInput":
            if name != partition_name:
                in_names.append(name)
        elif alloc.kind == "ExternalOutput":
            shape = tuple(alloc.tensor_shape)
            dtype = _mybir.dt.np(alloc.dtype)
            out_names.append(name)
            out_avals.append(jax.core.ShapedArray(shape, dtype))
            zero_outs.append(np.zeros(shape, dtype))
    n_params = len(in_names)
    all_names = in_names + out_names
    if partition_name is not None:
        all_names = all_names + [partition_name]
    donate = tuple(range(n_params, n_params + len(out_names)))

    def _body(*args):
        operands = list(args)
        if partition_name is not None:
            operands.append(bass2jax.partition_id_tensor())
        outs = bass2jax._bass_exec_p.bind(
            *operands, out_avals=tuple(out_avals), in_names=tuple(all_names),
            out_names=tuple(out_names), lowering_input_output_aliases=(),
            sim_require_finite=True, sim_require_nnan=True, nc=nc)
        return tuple(outs)

    mesh = Mesh(np.asarray(jax.devices()[:N_CORES]), ("core",))
    in_specs = (PartitionSpec("core"),) * (n_params + len(out_names))
    out_specs = (PartitionSpec("core"),) * len(out_names)
    sharded = jax.jit(
        shard_map(_body, mesh=mesh, in_specs=in_specs, out_specs=out_specs,
                  check_rep=False),
        donate_argnums=donate, keep_unused=True)
    concat_zeros = [np.zeros((N_CORES * z.shape[0], *z.shape[1:]), z.dtype)
                    for z in zero_outs]

    def run(named_inputs):  # dict name -> [B-like, ...] concatenated arrays
        args = [named_inputs[n] for n in in_names]
        outs = sharded(*args, *[z.copy() for z in concat_zeros])
        return np.asarray(outs[out_names.index("out")])

    return run


def _run(data: np.ndarray, trace: bool = False):
    data = np.ascontiguousarray(data, dtype=np.float32)
    assert data.shape == (B, A, F), data.shape
    dt_uniform, k_red, k_exact = _params_for(data)
    nc = _get_nc(dt_uniform, k_red, k_exact)
    in_maps = _in_maps_for(data, dt_uniform, k_red, k_exact)
    res = run_bass_kernel_spmd(nc, in_maps, core_ids=list(range(N_CORES)),
                               trace=trace)
    full = np.concatenate([res.results[c]["out"] for c in range(N_CORES)],
                          axis=0)
    return full, res


def kernel(data: np.ndarray) -> np.ndarray:
    data = np.ascontiguousarray(data, dtype=np.float32)
    assert data.shape == (B, A, F), data.shape
    dt_uniform, k_red, k_exact = _params_for(data)
    key = ("runner", dt_uniform, k_red, k_exact)
    if key not in _cache:
        _cache[key] = _make_runner(_get_nc(dt_uniform, k_red, k_exact))
    d2, dv, co = _host_inputs(data, dt_uniform, k_exact)
    named = {"data2": d2, "dv": dv}
    if co is not None:
        named["coef"] = np.ascontiguousarray(
            np.broadcast_to(co[None, :, :], (N_CORES, PB, co.shape[1]))
        ).reshape(N_CORES * PB, -1)
    return _cache[key](named).astype(np.float32)


# revision 4
# speedup vs baseline: 1.0809x; 1.0099x over previous
"""Trainium2 Bass kernel for nn_BackupBarrierCBF — v3.

v2 (121.8us) + rollout restructure:
  - Serial ACT-Tanh chain only until |v| provably <= 0.2 (k_exact ~ 18
    steps from vmax~15; host-verified on a dense grid). The remaining
    TAIL = 50-k_exact slots are VECTORIZED: v_{k+m} = w*FV_m(w^2) and
    S_{k+m} = S_k + w*FS_m(w^2) with per-slot deg-1 polynomial fits in
    u = w^2 computed on host (max fit err ~1.3e-3 on v, 7e-4 on S —
    fp16-noise level). Coefficients ship pre-broadcast as an extra
    fp16 DMA input (hidden behind the serial phase).
  - Tail angles |v|<=0.2: no range reduction, no abs pass for cos.
  - Constants phase: single batched range-reduce + one Sin over [4A]
    (shift-fold trick for the cos halves), one Sqrt over [2A]; all DVE
    prep interleaved into the chain's DVE gaps; ACT ops after the chain
    (no Sin/Tanh table thrash).
  - STH convert on DVE post-chain (off the ACT trig path).
Distance phase identical to v2 (fp16 t-major, 2x DVE).
"""
import numpy as np
import concourse.bass as bass
import concourse.bacc as bacc
import concourse.tile as tile
from concourse import mybir
from concourse.bass_utils import run_bass_kernel_spmd

F32 = mybir.dt.float32
F16 = mybir.dt.float16
I32 = mybir.dt.int32
OP = mybir.AluOpType
ACT = mybir.ActivationFunctionType

B, A, F = 1024, 64, 15
N_CORES = 8
PB = B // N_CORES
T = 50
NC2 = 2 * A
NT = T * A
TWO_PI = float(2.0 * np.pi)
MAGIC = float(1.5 * 2 ** 23)   # fp32 round-to-nearest-even bias

_cache: dict = {}


def _ap(t: bass.AP, extra_offset: int, free_dims: list) -> bass.AP:
    return bass.AP(tensor=t.tensor, offset=t.offset + extra_offset,
                   ap=[list(t.ap[0])] + [list(d) for d in free_dims])


def _build(dt_uniform, k_red, k_exact):
    tail = T - k_exact
    m0 = min(15, tail)          # deg-1 V slots; V deg-0 beyond, S deg-0 all
    ncoef = (m0 + 2 * tail) * NC2   # [AV1(m0) | BV1(m0) | AV0(tail-m0) | AS0(tail)]
    nc = bacc.Bacc("TRN2", target_bir_lowering=False)
    # field-major input [PB, F*A]: field k occupies cols [k*A, (k+1)*A)
    data = nc.dram_tensor("data2", [PB, F * A], F32, kind="ExternalInput")
    # duplicate of the two v columns; tiny DMA so the chain starts early
    dvin = nc.dram_tensor("dv", [PB, 2 * A], F32, kind="ExternalInput")
    coef = (nc.dram_tensor("coef", [PB, ncoef], F16,
                           kind="ExternalInput") if tail > 0 else None)
    out = nc.dram_tensor("out", [PB, A], F32, kind="ExternalOutput")

    with tile.TileContext(nc) as tc:
        with tc.tile_pool(name="pool", bufs=1) as pool:
            Dv = pool.tile([PB, 2 * A], F32)
            nc.sync.dma_start(out=Dv[:], in_=dvin[:])
            # split the main input across two queues: fields 0..7 (args,
            # p0 — feed the prep chain) land first; 8..14 follow in
            # parallel from the Pool queue
            D = pool.tile([PB, F * A], F32)
            nc.sync.dma_start(out=_ap(D, 0, [[1, 8 * A]]),
                              in_=_ap(data[:], 0, [[1, 8 * A]]))
            nc.gpsimd.dma_start(out=_ap(D, 8 * A, [[1, 7 * A]]),
                                in_=_ap(data[:], 8 * A, [[1, 7 * A]]))
            CO = None
            if tail > 0:
                CO = pool.tile([PB, ncoef], F16)
                nc.gpsimd.dma_start(out=CO[:], in_=coef[:])

            def fld(k):
                return _ap(D, k * A, [[1, A]])

            halfpi = pool.tile([PB, 1], F32)
            cons = pool.tile([PB, 12, A], F32)
            consh = pool.tile([PB, 12, A], F16)
            scr = pool.tile([PB, 10, A], F32)
            ki = pool.tile([PB, 4, A], I32)

            def c(i):
                return _ap(cons, i * A, [[1, A]])

            def cbh(i):
                return _ap(consh, i * A, [[0, T], [1, A]])

            def s(i):
                return _ap(scr, i * A, [[1, A]])

            C_P0X, C_P0Y = 0, 1
            C_D1, C_D2, C_D3, C_K2Y = 2, 3, 4, 5
            C_CEDT, C_SEDT, C_CADT, C_SADT = 6, 7, 8, 9
            C_RE, C_RA = 10, 11

            # ---- constants prep (pure DVE), interleaved into the chain ----
            # Batched sincos args: scr rows 0..3 hold args for
            # [cos(th_e), sin(th_e), cos(th_a), sin(th_a)] -> cons 6..9.
            prep = []
            S4 = _ap(scr, 0, [[1, 4 * A]])
            KI4 = _ap(ki, 0, [[1, 4 * A]])
            prep.append(lambda: nc.vector.memset(halfpi[:], float(np.pi / 2)))
            prep.append(lambda: nc.vector.tensor_copy(
                out=_ap(scr, 0, [[A, 2], [1, A]]),
                in_=_ap(D, 3 * A, [[0, 2], [1, A]])))
            prep.append(lambda: nc.vector.tensor_copy(
                out=_ap(scr, 2 * A, [[A, 2], [1, A]]),
                in_=_ap(D, 7 * A, [[0, 2], [1, A]])))
            # sc = th/2pi (+0.25 on the cos rows 0 and 2), then magic-number
            # round-to-nearest (adding 1.5*2^23 snaps the mantissa to ulp=1)
            prep.append(lambda: nc.vector.tensor_scalar(
                out=S4, in0=S4, scalar1=1.0 / TWO_PI, scalar2=0.0,
                op0=OP.mult, op1=OP.add))
            prep.append(lambda: nc.vector.tensor_scalar(
                out=_ap(scr, 0, [[2 * A, 2], [1, A]]),
                in0=_ap(scr, 0, [[2 * A, 2], [1, A]]),
                scalar1=1.0, scalar2=0.25, op0=OP.mult, op1=OP.add))
            prep.append(lambda: nc.vector.tensor_scalar(
                out=S4, in0=S4, scalar1=MAGIC, scalar2=MAGIC,
                op0=OP.add, op1=OP.subtract))
            # subtract the shift back on cos rows: k - 0.25
            prep.append(lambda: nc.vector.tensor_scalar(
                out=_ap(scr, 0, [[2 * A, 2], [1, A]]),
                in0=_ap(scr, 0, [[2 * A, 2], [1, A]]),
                scalar1=1.0, scalar2=-0.25, op0=OP.mult, op1=OP.add))
            # arg = th - 2pi*k  (th re-read strided)
            prep.append(lambda: nc.vector.scalar_tensor_tensor(
                out=_ap(scr, 0, [[1, A]]), in0=_ap(scr, 0, [[1, A]]),
                scalar=-TWO_PI, in1=fld(3), op0=OP.mult, op1=OP.add))
            prep.append(lambda: nc.vector.scalar_tensor_tensor(
                out=_ap(scr, A, [[1, A]]), in0=_ap(scr, A, [[1, A]]),
                scalar=-TWO_PI, in1=fld(3), op0=OP.mult, op1=OP.add))
            prep.append(lambda: nc.vector.scalar_tensor_tensor(
                out=_ap(scr, 2 * A, [[1, A]]), in0=_ap(scr, 2 * A, [[1, A]]),
                scalar=-TWO_PI, in1=fld(7), op0=OP.mult, op1=OP.add))
            prep.append(lambda: nc.vector.scalar_tensor_tensor(
                out=_ap(scr, 3 * A, [[1, A]]), in0=_ap(scr, 3 * A, [[1, A]]),
                scalar=-TWO_PI, in1=fld(7), op0=OP.mult, op1=OP.add))
            # sumsq for r_e, r_a into scr rows 4 (ego) and 5 (agent)
            prep.append(lambda: nc.vector.tensor_mul(
                out=s(4), in0=fld(8), in1=fld(8)))
            prep.append(lambda: nc.vector.tensor_mul(
                out=s(6), in0=fld(9), in1=fld(9)))
            prep.append(lambda: nc.vector.tensor_add(
                out=s(4), in0=s(4), in1=s(6)))
            prep.append(lambda: nc.vector.tensor_mul(
                out=s(5), in0=fld(11), in1=fld(11)))
            prep.append(lambda: nc.vector.tensor_mul(
                out=s(6), in0=fld(12), in1=fld(12)))
            prep.append(lambda: nc.vector.tensor_add(
                out=s(5), in0=s(5), in1=s(6)))
            # d1/d2/p0
            prep.append(lambda: nc.vector.tensor_sub(
                out=s(6), in0=fld(9), in1=fld(8)))
            prep.append(lambda: nc.vector.tensor_scalar_mul(
                out=c(C_D1), in0=s(6), scalar1=0.5))
            prep.append(lambda: nc.vector.tensor_sub(
                out=s(7), in0=fld(12), in1=fld(11)))
            prep.append(lambda: nc.vector.tensor_scalar_mul(
                out=c(C_D2), in0=s(7), scalar1=0.5))
            prep.append(lambda: nc.vector.tensor_sub(
                out=c(C_P0X), in0=fld(4), in1=fld(0)))
            prep.append(lambda: nc.vector.tensor_sub(
                out=c(C_P0Y), in0=fld(5), in1=fld(1)))

            # ---------------- rollout: serial head ----------------
            VT = pool.tile([PB, (k_exact + 1) * NC2], F32)
            ST = pool.tile([PB, (k_exact + 1) * NC2], F32)
            STH = pool.tile([PB, T * NC2], F16)
            G = pool.tile([PB, NC2], F32)

            def vslot(j):
                if j == 0:
                    return _ap(Dv, 0, [[A, 2], [1, A]])
                return _ap(VT, j * NC2, [[1, NC2]])

            def stslot(k):
                return _ap(ST, k * NC2, [[1, NC2]])

            nc.vector.tensor_copy(out=stslot(0), in_=vslot(0))

            NDT2 = None
            if dt_uniform is None:
                NDT2 = pool.tile([PB, NC2], F32)
                nc.vector.tensor_scalar_mul(
                    out=NDT2[:], in0=_ap(D, 14 * A, [[0, 2], [1, A]]),
                    scalar1=-9.0)

            # Interleave preps into the chain from step 4 on (2 per step):
            # by then the big D DMA has landed, so a prep never stalls the
            # chain's STTs (emitting preps after the loop would let the
            # scheduler place all of them — waiting on D — ahead of STT1).
            pi = 0
            for j in range(1, k_exact + 1):
                nc.scalar.activation(out=G[:], in_=vslot(j - 1),
                                     func=ACT.Tanh, scale=2.0)
                if dt_uniform is None:
                    nc.vector.tensor_mul(out=G[:], in0=G[:], in1=NDT2[:])
                    nc.vector.tensor_add(out=vslot(j), in0=vslot(j - 1),
                                         in1=G[:])
                else:
                    nc.vector.scalar_tensor_tensor(
                        out=vslot(j), in0=G[:], scalar=-9.0 * float(dt_uniform),
                        in1=vslot(j - 1), op0=OP.mult, op1=OP.add)
                nc.vector.tensor_add(out=stslot(j), in0=stslot(j - 1),
                                     in1=vslot(j))
                if j >= 4:
                    for _ in range(2):
                        if pi < len(prep):
                            prep[pi]()
                            pi += 1
            while pi < len(prep):
                prep[pi]()
                pi += 1

            # ---- range-reduce head angle slots 1..k_red (in place) ----
            # post-chain: the chain window's DVE slack is already filled
            # by the preps (v9 showed adding this mid-loop stretches it)
            if k_red > 0:
                MS = pool.tile([PB, NC2 * k_red], F32, tag="tPXY")
                red_view = _ap(VT, NC2, [[1, NC2 * k_red]])
                nc.vector.tensor_scalar(out=MS[:], in0=red_view,
                                        scalar1=1.0 / TWO_PI, scalar2=MAGIC,
                                        op0=OP.mult, op1=OP.add)
                nc.vector.tensor_scalar(out=MS[:], in0=MS[:],
                                        scalar1=MAGIC, scalar2=-TWO_PI,
                                        op0=OP.subtract, op1=OP.mult)
                nc.vector.tensor_add(out=red_view, in0=red_view, in1=MS[:])
            # chain-end gate: z = 0 * v_k, then += 0 on the ACT-const
            # inputs so Sqrt/Sin (and their table loads) cannot be
            # scheduled into the middle of the Tanh chain.
            zg = pool.tile([PB, 1], F32)
            nc.vector.tensor_scalar_mul(out=zg[:],
                                        in0=_ap(VT, k_exact * NC2, [[1, 1]]),
                                        scalar1=0.0)
            nc.vector.tensor_add(out=_ap(scr, 0, [[NC2, 2], [1, NC2]]),
                                 in0=_ap(scr, 0, [[NC2, 2], [1, NC2]]),
                                 in1=_ap(zg, 0, [[0, 2], [0, NC2]]))
            nc.vector.tensor_add(out=_ap(scr, 4 * A, [[A, 2], [1, A]]),
                                 in0=_ap(scr, 4 * A, [[A, 2], [1, A]]),
                                 in1=_ap(zg, 0, [[0, 2], [0, A]]))

            # ---- ACT constants: one Sqrt over [2A], one Sin over [4A] ----
            nc.scalar.activation(out=_ap(cons, C_RE * A, [[1, 2 * A]]),
                                 in_=_ap(scr, 4 * A, [[1, 2 * A]]),
                                 func=ACT.Sqrt, scale=0.25)
            # gate the Sin batch on the Sqrt output so the scheduler can't
            # interleave Sqrt between the Sin-table trig passes
            zg2 = pool.tile([PB, 1], F32)
            nc.vector.tensor_scalar_mul(out=zg2[:],
                                        in0=_ap(cons, C_RE * A, [[1, 1]]),
                                        scalar1=0.0)
            nc.vector.tensor_add(out=_ap(scr, 0, [[NC2, 2], [1, NC2]]),
                                 in0=_ap(scr, 0, [[NC2, 2], [1, NC2]]),
                                 in1=_ap(zg2, 0, [[0, 2], [0, NC2]]))
            nc.scalar.activation(out=_ap(cons, C_CEDT * A, [[1, 4 * A]]),
                                 in_=S4, func=ACT.Sin)
            # finish constants on DVE (cheap, off chain)
            for i in (C_CADT, C_SADT, C_CEDT, C_SEDT):
                nc.vector.tensor_mul(out=c(i), in0=c(i), in1=fld(14))
            nc.vector.scalar_tensor_tensor(
                out=s(8), in0=fld(9), scalar=0.5, in1=c(C_RA),
                op0=OP.mult, op1=OP.add)          # k1y
            nc.vector.scalar_tensor_tensor(
                out=c(C_K2Y), in0=fld(12), scalar=0.5, in1=c(C_RE),
                op0=OP.mult, op1=OP.add)          # k2y
            nc.vector.tensor_sub(out=c(C_D3), in0=c(C_K2Y), in1=s(8))
            nc.vector.tensor_copy(out=_ap(consh, 0, [[1, 12 * A]]),
                                  in_=_ap(cons, 0, [[1, 12 * A]]))

            # ---------------- vectorized tail ----------------
            VTT = pool.tile([PB, max(1, tail) * NC2], F16)
            wv = pool.tile([PB, NC2], F16)
            uv = pool.tile([PB, NC2], F16)
            s20h = pool.tile([PB, NC2], F16)
            if tail > 0:
                nc.vector.tensor_copy(out=wv[:], in_=vslot(k_exact))
                nc.vector.tensor_mul(out=uv[:], in0=vslot(k_exact),
                                     in1=vslot(k_exact))
                nc.vector.tensor_copy(out=s20h[:], in_=stslot(k_exact))

                def bcast(tl, n):
                    return _ap(tl, 0, [[0, n], [1, NC2]])

                # angles, deg-1 slots 0..m0: VTT = (BV1*u + AV1) * w
                O_BV1, O_AV0, O_AS0 = m0 * NC2, 2 * m0 * NC2, \
                    (m0 + tail) * NC2
                V1a = _ap(VTT, 0, [[1, m0 * NC2]])
                V1b = _ap(VTT, 0, [[NC2, m0], [1, NC2]])
                nc.vector.tensor_mul(out=V1b,
                                     in0=_ap(CO, O_BV1,
                                             [[NC2, m0], [1, NC2]]),
                                     in1=bcast(uv, m0))
                nc.vector.tensor_add(out=V1a, in0=V1a,
                                     in1=_ap(CO, 0, [[1, m0 * NC2]]))
                nc.vector.tensor_mul(out=V1b, in0=V1b, in1=bcast(wv, m0))
                # angles, deg-0 slots m0..tail: VTT = AV0 * w
                if tail > m0:
                    nc.vector.tensor_mul(
                        out=_ap(VTT, m0 * NC2, [[NC2, tail - m0], [1, NC2]]),
                        in0=_ap(CO, O_AV0, [[NC2, tail - m0], [1, NC2]]),
                        in1=bcast(wv, tail - m0))
                # cumsum, deg-0 all slots: STH[k_exact..] = AS0 * w + S_k
                SHT2 = _ap(STH, k_exact * NC2, [[NC2, tail], [1, NC2]])
                nc.vector.tensor_mul(out=SHT2,
                                     in0=_ap(CO, O_AS0,
                                             [[NC2, tail], [1, NC2]]),
                                     in1=bcast(wv, tail))
                nc.vector.tensor_add(out=SHT2, in0=SHT2, in1=bcast(s20h, tail))

            # fp16 convert of the head cumsum (slots 0..k_exact-1) on ACT
            # (it has slack here; DVE is the critical path)
            nc.scalar.activation(out=_ap(STH, 0, [[1, k_exact * NC2]]),
                                 in_=_ap(ST, 0, [[1, k_exact * NC2]]),
                                 func=ACT.Copy)

            # ---------------- trig ----------------
            SINV = pool.tile([PB, T * NC2], F16)
            COSV = pool.tile([PB, T * NC2], F16)
            ang = _ap(VT, NC2, [[1, k_exact * NC2]])
            nc.scalar.activation(out=_ap(SINV, 0, [[1, k_exact * NC2]]),
                                 in_=ang, func=ACT.Sin)
            nc.scalar.activation(out=ang, in_=ang, func=ACT.Abs)
            nc.scalar.activation(out=_ap(COSV, 0, [[1, k_exact * NC2]]),
                                 in_=ang, func=ACT.Sin, bias=halfpi[:],
                                 scale=-1.0)
            if tail > 0:
                VTT1 = _ap(VTT, 0, [[1, tail * NC2]])
                nc.scalar.activation(out=_ap(SINV, k_exact * NC2,
                                             [[1, tail * NC2]]),
                                     in_=VTT1, func=ACT.Sin)
                nc.scalar.activation(out=_ap(COSV, k_exact * NC2,
                                             [[1, tail * NC2]]),
                                     in_=VTT1, func=ACT.Sin, bias=halfpi[:],
                                     scale=-1.0)

            # ---------------- distance phase (fp16, t-major) -----------
            def half(tl, off):
                return _ap(tl, off, [[NC2, T], [1, A]])

            SEh, SAh = half(STH, 0), half(STH, A)
            SINE, SINA = half(SINV, 0), half(SINV, A)
            COSE, COSA = half(COSV, 0), half(COSV, A)

            PXY = pool.tile([PB, 2 * NT], F16, tag="tPXY2")
            SCR = pool.tile([PB, 2 * NT], F16, tag="tSCR2")
            R12 = pool.tile([PB, 2 * NT], F16)
            R34 = pool.tile([PB, 2 * NT], F16)

            def v2(tl, off):
                return _ap(tl, off, [[A, T], [1, A]])

            def v1(tl, off):
                return _ap(tl, off, [[1, NT]])

            PX2, PY2 = v2(PXY, 0), v2(PXY, NT)
            S1_2, S2_2 = v2(SCR, 0), v2(SCR, NT)
            S1_1, S2_1 = v1(SCR, 0), v1(SCR, NT)
            R1X2, R1Y2 = v2(R12, 0), v2(R12, NT)
            R2X2, R2Y2 = v2(R34, 0), v2(R34, NT)
            R1X1, R1Y1 = v1(R12, 0), v1(R12, NT)
            R2X1, R2Y1 = v1(R34, 0), v1(R34, NT)

            nc.vector.tensor_mul(out=S1_2, in0=SAh, in1=cbh(C_CADT))
            nc.vector.tensor_add(out=S1_2, in0=S1_2, in1=cbh(C_P0X))
            nc.vector.tensor_mul(out=S2_2, in0=SEh, in1=cbh(C_CEDT))
            nc.vector.tensor_sub(out=PX2, in0=S1_2, in1=S2_2)
            nc.vector.tensor_mul(out=S1_2, in0=SAh, in1=cbh(C_SADT))
            nc.vector.tensor_add(out=S1_2, in0=S1_2, in1=cbh(C_P0Y))
            nc.vector.tensor_mul(out=S2_2, in0=SEh, in1=cbh(C_SEDT))
            nc.vector.tensor_sub(out=PY2, in0=S1_2, in1=S2_2)

            nc.vector.tensor_mul(out=R1X2, in0=SINE, in1=PY2)
            nc.vector.tensor_mul(out=R1Y2, in0=SINE, in1=PX2)
            nc.vector.tensor_mul(out=R2X2, in0=SINA, in1=PY2)
            nc.vector.tensor_mul(out=R2Y2, in0=SINA, in1=PX2)
            nc.vector.tensor_mul(out=S1_2, in0=COSE, in1=PX2)
            nc.vector.tensor_add(out=R1X1, in0=R1X1, in1=S1_1)
            nc.vector.tensor_mul(out=S2_2, in0=COSE, in1=PY2)
            nc.vector.tensor_sub(out=R1Y1, in0=S2_1, in1=R1Y1)
            nc.vector.tensor_mul(out=S1_2, in0=COSA, in1=PX2)
            nc.vector.tensor_add(out=R2X1, in0=R2X1, in1=S1_1)
            nc.vector.tensor_mul(out=S2_2, in0=COSA, in1=PY2)
            nc.vector.tensor_sub(out=R2Y1, in0=R2Y1, in1=S2_1)

            for R in (R1X1, R1Y1, R2X1, R2Y1):
                nc.scalar.activation(out=R, in_=R, func=ACT.Abs)
            nc.vector.tensor_add(out=R1X2, in0=R1X2, in1=cbh(C_D1))
            nc.vector.tensor_tensor(out=R1X1, in0=R1X1, in1=R1Y1, op=OP.max)
            nc.vector.tensor_add(out=R2X2, in0=R2X2, in1=cbh(C_D2))
            nc.vector.tensor_tensor(out=R2X1, in0=R2X1, in1=R2Y1, op=OP.max)
            nc.vector.tensor_add(out=R1X2, in0=R1X2, in1=cbh(C_D3))
            nc.vector.tensor_tensor(out=R1X1, in0=R1X1, in1=R2X1, op=OP.max)

            W = SCR
            nc.vector.tensor_tensor(out=_ap(W, 0, [[1, 25 * A]]),
                                    in0=_ap(R12, 0, [[1, 25 * A]]),
                                    in1=_ap(R12, 25 * A, [[1, 25 * A]]),
                                    op=OP.min)
            n = 25
            while n > 1:
                h = n // 2
                if n % 2:
                    nc.vector.tensor_tensor(
                        out=_ap(W, 0, [[1, A]]), in0=_ap(W, 0, [[1, A]]),
                        in1=_ap(W, (n - 1) * A, [[1, A]]), op=OP.min)
                nc.vector.tensor_tensor(out=_ap(W, 0, [[1, h * A]]),
                                        in0=_ap(W, 0, [[1, h * A]]),
                                        in1=_ap(W, h * A, [[1, h * A]]),
                                        op=OP.min)
                n = h

            H = pool.tile([PB, A], F32)
            nc.vector.tensor_sub(out=H[:], in0=_ap(W, 0, [[1, A]]),
                                 in1=c(C_K2Y))
            OUTT = pool.tile([PB, A], F32)
            nc.scalar.activation(out=H[:], in_=H[:], func=ACT.Tanh, scale=0.1)
            nc.vector.tensor_scalar_mul(out=OUTT[:], in0=H[:], scalar1=5.0)
            nc.sync.dma_start(out=out[:], in_=OUTT[:])

    nc.compile()
    return nc


def _get_nc(dt_uniform, k_red, k_exact):
    key = ("nc", dt_uniform, k_red, k_exact)
    if key not in _cache:
        _cache[key] = _build(dt_uniform, k_red, k_exact)
    return _cache[key]


def _fit_tail_coefs(dt, k_exact, tail):
    """Tail fits in u=w^2: v_{k+m} = w*(AV1_m + BV1_m*u) for m<m0 and
    w*AV0_m beyond (deg-0 err < 1e-3 there); (S_{k+m}-S_k) = w*AS0_m for
    all m (deg-0 S err ~6e-3 is scaled by cadt~0.1 downstream, so it is
    ~6e-4 in position). Layout [AV1(m0)|BV1(m0)|AV0(tail-m0)|AS0(tail)],
    each row expanded to NC2 fp16 values."""
    m0 = min(15, tail)
    w = np.linspace(-0.21, 0.21, 20001)
    w = w[np.abs(w) > 1e-9]
    u = w * w
    basis1 = np.stack([np.ones_like(u), u], axis=1)
    x = w.copy()
    AV1 = np.zeros(m0); BV1 = np.zeros(m0)
    AV0 = np.zeros(tail - m0); AS0 = np.zeros(tail)
    pref = np.zeros_like(w)
    for m in range(tail):
        x = x - 9.0 * dt * np.tanh(2.0 * x)   # phi_{m+1}
        if m < m0:
            cv, *_ = np.linalg.lstsq(basis1, x / w, rcond=None)
            AV1[m], BV1[m] = cv
        else:
            AV0[m - m0] = np.mean(x / w)
        if m > 0:
            AS0[m] = np.mean(pref / w)
        pref = pref + x
    co = np.concatenate([AV1, BV1, AV0, AS0])    # [m0 + 2*tail]
    co = np.repeat(co[:, None], NC2, axis=1)     # [., NC2]
    return np.ascontiguousarray(co.reshape(1, -1).astype(np.float16))


def _params_for(data: np.ndarray):
    dt = data[..., 14]
    dt0 = float(dt.flat[0])
    dt_uniform = dt0 if bool(np.all(dt == dt0)) else None
    vmax = float(np.abs(data[..., [2, 6]]).max())
    dt_min = float(dt.min())
    dt_max = float(dt.max())
    shrink = 9.0 * dt_min * 0.9997
    if 9.0 * dt_max > np.pi or shrink <= 1e-6:
        k_red = T
    else:
        k_red = int(min(T, max(0, np.ceil((vmax - np.pi) / shrink) + 1)))
    # k_exact: steps (verified on a dense grid) until |v| <= 0.2
    if dt_uniform is None:
        k_exact = T
    else:
        g = np.linspace(0.0, vmax + 1e-3, 200001)
        k_exact = T
        for j in range(1, T + 1):
            g = g - 9.0 * dt_uniform * np.tanh(2.0 * g)
            if np.abs(g).max() <= 0.2:
                k_exact = j
                break
    k_exact = int(min(T, max(k_exact, k_red, 1)))
    return dt_uniform, k_red, k_exact


def _host_inputs(data, dt_uniform, k_exact):
    """data [B, A, F] -> field-major data2 [B, F*A], dv [B, 2A], coef."""
    d2 = np.ascontiguousarray(data.transpose(0, 2, 1)).reshape(B, F * A)
    dv = np.ascontiguousarray(data[..., [2, 6]].transpose(0, 2, 1)
                              ).reshape(B, 2 * A)
    tail = T - k_exact
    co = None
    if tail > 0:
        key = ("coef", dt_uniform, k_exact)
        if key not in _cache:
            _cache[key] = _fit_tail_coefs(dt_uniform, k_exact, tail)
        co = np.ascontiguousarray(
            np.broadcast_to(_cache[key], (PB, _cache[key].shape[1])))
    return d2, dv, co


def _in_maps_for(data, dt_uniform, k_red, k_exact):
    d2, dv, co = _host_inputs(data, dt_uniform, k_exact)
    in_maps = []
    for c in range(N_CORES):
        m = {"data2": d2[c * PB:(c + 1) * PB], "dv": dv[c * PB:(c + 1) * PB]}
        if co is not None:
            m["coef"] = co
        in_maps.append(m)
    return in_maps


def _make_runner(nc):
    import jax
    from jax.sharding import Mesh, PartitionSpec
    from jax.experimental.shard_map import shard_map
    from concourse import bass2jax, mybir as _mybir

    bass2jax.install_neuronx_cc_hook()
    partition_name = (nc.partition_id_tensor.name
                      if nc.partition_id_tensor else None)
    in_names, out_names, out_avals, zero_outs = [], [], [], []
    for alloc in nc.m.functions[0].allocations:
        if not isinstance(alloc, _mybir.MemoryLocationSet):
            continue
        name = alloc.memorylocations[0].name
        if alloc.kind == "ExternalInput":
            if name != partition_name:
                in_names.append(name)
        elif alloc.kind == "ExternalOutput":
            shape = tuple(alloc.tensor_shape)
            dtype = _mybir.dt.np(alloc.dtype)
            out_names.append(name)
            out_avals.append(jax.core.ShapedArray(shape, dtype))
            zero_outs.append(np.zeros(shape, dtype))
    n_params = len(in_names)
    all_names = in_names + out_names
    if partition_name is not None:
        all_names = all_names + [partition_name]
    donate = tuple(range(n_params, n_params + len(out_names)))

    def _body(*args):
        operands = list(args)
        if partition_name is not None:
            operands.append(bass2jax.partition_id_tensor())
        outs = bass2jax._bass_exec_p.bind(
            *operands, out_avals=tuple(out_avals), in_names=tuple(all_names),
            out_names=tuple(out_names), lowering_input_output_aliases=(),
            sim_require_finite=True, sim_require_nnan=True, nc=nc)
        return tuple(outs)

    mesh = Mesh(np.asarray(jax.devices()[:N_CORES]), ("core",))
    in_specs = (PartitionSpec("core"),) * (n_params + len(out_names))
    out_specs = (PartitionSpec("core"),) * len(out_names)
    sharded = jax.jit(
        shard_map(_body, mesh=mesh, in_specs=in_specs, out_specs=out_specs,
                  check_rep=False),
        donate_argnums=donate, keep_unused=True)
    concat_zeros = [np.zeros((N_CORES * z.shape[0], *z.shape[1:]), z.dtype)
                    for z in zero_outs]

    def run(named_inputs):  # dict name -> [B-like, ...] concatenated arrays
        args = [named_inputs[n] for n in in_names]
        outs = sharded(*args, *[z.copy() for z in concat_zeros])
        return np.asarray(outs[out_names.index("out")])

    return run


def _run(data: np.ndarray, trace: bool = False):
    data = np.ascontiguousarray(data, dtype=np.float32)
    assert data.shape == (B, A, F), data.shape
    dt_uniform, k_red, k_exact = _params_for(data)
    nc = _get_nc(dt_uniform, k_red, k_exact)
    in_maps = _in_maps_for(data, dt_uniform, k_red, k_exact)
    res = run_bass_kernel_spmd(nc, in_maps, core_ids=list(range(N_CORES)),
                               trace=trace)
    full = np.concatenate([res.results[c]["out"] for c in range(N_CORES)],
                          axis=0)
    return full, res


def kernel(data: np.ndarray) -> np.ndarray:
    data = np.ascontiguousarray(data, dtype=np.float32)
    assert data.shape == (B, A, F), data.shape
    dt_uniform, k_red, k_exact = _params_for(data)
    key = ("runner", dt_uniform, k_red, k_exact)
    if key not in _cache:
        _cache[key] = _make_runner(_get_nc(dt_uniform, k_red, k_exact))
    d2, dv, co = _host_inputs(data, dt_uniform, k_exact)
    named = {"data2": d2, "dv": dv}
    if co is not None:
        named["coef"] = np.ascontiguousarray(
            np.broadcast_to(co[None, :, :], (N_CORES, PB, co.shape[1]))
        ).reshape(N_CORES * PB, -1)
    return _cache[key](named).astype(np.float32)
